# revision 1
# baseline (speedup 1.0000x reference)
"""Multi-head attention (dense_transformer) on 8 TRN2 NeuronCores.

Sharding: 2-way data parallel over batch x 4-way tensor parallel over heads.
Core c handles batch b=c//4 and heads {4g..4g+3} where g=c%4 (4 heads, 256
channels per core; channels of head h are qw columns {hd*16+h}).

Architecture (v2, "transposed scores"):
  phase 1: Q^T/K^T projections in [ch, s] layout via 3-term bf16 splits
           (pseudo-fp32, needed because softmax here is a near-argmax: score
           std ~256, so score errors flip the max). V is projected DIRECTLY
           into [s, ch] layout with single-pass f32r matmuls (V error is
           linear in the output -> 11-bit mantissa is plenty), with a ones
           column appended per head so AV also produces the softmax sums.
  phase 2: scores computed TRANSPOSED: scT[k,q] = K^T-chunk (stationary) x
           Q^T (moving), 3-term bf16. Per 512-wide q block: drain chunks to
           SBUF, running column-max on gpsimd (Pool engine, otherwise idle),
           one partition_all_reduce(max) -> bias replicated on all
           partitions, DVE subtract + ACT exp -> expT bf16, then
           AV = Vhat-chunk (stationary [128,65]) x expT (moving) accumulates
           O^T[ch,q] AND l[q] in PSUM with no transposes at all.
           Normalize = reciprocal of l + gpsimd partition_broadcast + the
           PSUM drain is a tensor_tensor multiply.
  phase 3: AllGather O^T across the 4 cores of the batch -> out-proj column
           slice (out^T = ow_perm^T @ merged^T, f32r) -> DMA out.

vs v1: no PE transposes (was 576 matmuls / ~97us), V projection 3x cheaper,
no separate normalize pass, no exp accum; PE stream is denser so it holds
the 2.4GHz p-state instead of 1.2GHz.
"""
import sys

sys.path.insert(0, "/opt/trn_rl_repo")

import numpy as np

import concourse.bass as bass
import concourse.mybir as mybir
import concourse.tile as tile
from concourse import bacc
from concourse import bass_isa
from concourse.bass_utils import run_bass_kernel_spmd

# ---- problem constants (hardcoded per harness contract) ----
B, S, D, HEADS = 2, 2048, 1024, 16
N_CORES = 8
GROUPS = 4                 # head-groups == cores per batch
HPC = HEADS // GROUPS      # heads per core (4)
HD = D // HEADS            # 64
CPC = HPC * HD             # channels per core (256)
P = 128
NCC = CPC // P             # col chunks per core (2)
DCH = D // P               # contraction chunks (8)
QB = 512                   # q block width (1 PSUM bank of f32)

f32 = mybir.dt.float32
f32r = mybir.dt.float32r
bf16 = mybir.dt.bfloat16

AX = mybir.AxisListType
EXP = mybir.ActivationFunctionType.Exp
MAXOP = mybir.AluOpType.max
SUB = mybir.AluOpType.subtract
MULT = mybir.AluOpType.mult
COPYF = mybir.ActivationFunctionType.Copy
LN = mybir.ActivationFunctionType.Ln

DEFAULT_CFG = dict(s=S)


def make_maskT(nc, maskT, mask_val=-1e10):
    """maskT[k, q] = 0 if q >= k else mask_val (transposed causal)."""
    sq = maskT.shape[0]
    nc.gpsimd.memset(maskT, mask_val)
    nc.gpsimd.affine_select(
        out=maskT,
        in_=maskT,
        compare_op=mybir.AluOpType.is_gt,
        fill=0.0,
        base=0,
        # keep mask_val where (k - q) > 0, else fill 0
        pattern=[[-1, sq]],
        channel_multiplier=1,
    )


def build_nc(s=S, dbg=False):
    assert s % QB == 0
    NQB = s // QB            # 512-wide q blocks
    NKC = s // P             # 128-wide k chunks
    KPB = QB // P            # k chunks per q block on the diagonal (4)
    VW = 65                  # V channels per (hp,h2) incl the ones column
    NH2 = NCC * 2            # head slots per core (4)

    nc = bacc.Bacc("TRN2", target_bir_lowering=False, debug=False,
                   num_devices=N_CORES)
    xT = nc.dram_tensor("xT", [D, s], f32, kind="ExternalInput").ap()
    wq = nc.dram_tensor("wq", [D, CPC], f32, kind="ExternalInput").ap()
    wk = nc.dram_tensor("wk", [D, CPC], f32, kind="ExternalInput").ap()
    wv = nc.dram_tensor("wv", [D, CPC], f32r, kind="ExternalInput").ap()
    wo = nc.dram_tensor("wo", [D, CPC], f32, kind="ExternalInput").ap()
    outT = nc.dram_tensor("outT", [NCC, P, s], f32, kind="ExternalOutput").ap()

    with tile.TileContext(nc) as tc:
        with (
            tc.tile_pool(name="cpool", bufs=1) as cpool,
            tc.tile_pool(name="wpool", bufs=1) as wpool,
            tc.tile_pool(name="big", bufs=1) as big,
            tc.tile_pool(name="stat", bufs=3) as stat,
            tc.tile_pool(name="ms", bufs=6) as ms,
            tc.tile_pool(name="op", bufs=2) as op,
            tc.tile_pool(name="dram", bufs=1, space="DRAM") as dpool,
        ):
            NQB_ = s // QB
            ag_in = {qb: dpool.tile([P, NCC, QB], bf16, tag=f"agi{qb}",
                                    name=f"agi{qb}")
                     for qb in range(NQB_ - 1)}
            ag_out = {qb: dpool.tile([GROUPS, P, NCC, QB], bf16,
                                     tag=f"ago{qb}", name=f"ago{qb}")
                      for qb in range(NQB_ - 1)}
            ag_in_h = {(NQB_ - 1, hp): dpool.tile([P, QB], bf16,
                                                  tag=f"agih{hp}",
                                                  name=f"agih{hp}")
                       for hp in range(NCC)}
            ag_out_h = {(NQB_ - 1, hp): dpool.tile([GROUPS, P, QB], bf16,
                                                   tag=f"agoh{hp}",
                                                   name=f"agoh{hp}")
                        for hp in range(NCC)}

            maskT = cpool.tile([P, P], f32, tag="maskT")
            make_maskT(nc, maskT[:])
            mbias = cpool.tile([P, 1], f32, tag="mbias")
            nc.gpsimd.memset(mbias[:], -6.0)

            woh = wpool.tile([P, DCH, CPC], bf16, tag="woh")
            wol = wpool.tile([P, DCH, CPC], bf16, tag="wol")
            wv_sb = wpool.tile([P, DCH, CPC], f32r, tag="wv")
            for di in range(DCH):
                nc.sync.dma_start(wv_sb[:, di, :], wv[di * P:(di + 1) * P, :])
            wsplit = {}
            for nm in ("q", "k"):
                wh = wpool.tile([P, DCH, CPC], bf16, tag=f"w{nm}h", name=f"w{nm}h")
                wl = wpool.tile([P, DCH, CPC], bf16, tag=f"w{nm}l", name=f"w{nm}l")
                wsplit[nm] = [wh, wl]
            with tc.tile_pool(name="wload", bufs=2) as wload:
                for nm, wdr in (("q", wq), ("k", wk), ("o", wo)):
                    wf = wload.tile([P, DCH, CPC], f32, tag="wf", name="wf")
                    for di in range(DCH):
                        nc.sync.dma_start(wf[:, di, :],
                                          wdr[di * P:(di + 1) * P, :])
                    wh, wl = ((woh, wol) if nm == "o" else wsplit[nm])
                    nc.vector.tensor_copy(wh[:], wf[:])
                    nc.vector.tensor_tensor(wl[:], wf[:], wh[:], SUB)

            QTh = big.tile([P, NCC, s], bf16, tag="QTh")
            QTl = big.tile([P, NCC, s], bf16, tag="QTl")
            KTh = big.tile([P, NCC, s], bf16, tag="KTh")
            KTl = big.tile([P, NCC, s], bf16, tag="KTl")
            # Vhat[k, :]: 4 groups of 65 cols: 64 V channels + a ones col
            Vsb = big.tile([P, NKC, NH2 * VW], bf16, tag="Vsb")
            OT = big.tile([P, NCC, s], bf16, tag="OT")

            for g in range(NH2):
                nc.gpsimd.memset(Vsb[:, :, g * VW + 64], 1.0)

            # ---------------- phase 1: projections ----------------
            with (
                tc.tile_pool(name="psp", bufs=1, space="PSUM") as psp,
                tc.tile_pool(name="psv", bufs=1, space="PSUM") as psv,
                tc.tile_pool(name="xs", bufs=5) as xs,
            ):
                for qb in range(NQB):
                    accs = {}
                    for nm in ("q", "k"):
                        for cc in range(NCC):
                            accs[nm, cc] = psp.tile([P, QB], f32,
                                                    tag=f"pp{nm}{cc}",
                                                    name=f"pp{nm}{cc}")
                    vacc = [psv.tile([P, CPC], f32, tag=f"pv{r}", name=f"pv{r}")
                            for r in range(KPB)]
                    for di in range(DCH):
                        xt = xs.tile([P, QB], f32, tag="xt", name="xt")
                        nc.sync.dma_start(
                            xt[:], xT[di * P:(di + 1) * P, qb * QB:(qb + 1) * QB])
                        xth = xs.tile([P, QB], bf16, tag="xth", name="xth")
                        xtl = xs.tile([P, QB], bf16, tag="xtl", name="xtl")
                        nc.vector.tensor_copy(xth[:], xt[:])
                        nc.vector.tensor_tensor(xtl[:], xt[:], xth[:], SUB)
                        xtr = xs.tile([P, QB], f32r, tag="xtr", name="xtr")
                        nc.any.tensor_copy(xtr[:], xt[:])
                        for nm in ("q", "k"):
                            wh, wl = wsplit[nm]
                            for cc in range(NCC):
                                csl = slice(cc * P, (cc + 1) * P)
                                terms = [(wh, xth), (wh, xtl), (wl, xth)]
                                for ti, (wt, xtt) in enumerate(terms):
                                    nc.tensor.matmul(
                                        accs[nm, cc][:], wt[:, di, csl], xtt[:],
                                        start=(di == 0 and ti == 0),
                                        stop=(di == DCH - 1 and ti == len(terms) - 1))
                        for r in range(KPB):
                            nc.tensor.matmul(
                                vacc[r][:], xtr[:, r * P:(r + 1) * P],
                                wv_sb[:, di, :],
                                start=(di == 0), stop=(di == DCH - 1))
                    sl = slice(qb * QB, (qb + 1) * QB)
                    for cc in range(NCC):
                        for hi_t, lo_t, ps in ((QTh, QTl, accs["q", cc]),
                                               (KTh, KTl, accs["k", cc])):
                            nc.any.tensor_copy(hi_t[:, cc, sl], ps[:])
                            nc.vector.tensor_tensor(lo_t[:, cc, sl], ps[:],
                                                    hi_t[:, cc, sl], SUB)
                    for r in range(KPB):
                        ki = qb * KPB + r
                        # strided dest: 4 groups of 64 V channels (skip ones col)
                        dst = Vsb[:, ki].rearrange("p (g w) -> p g w", w=VW)[:, :, 0:64]
                        nc.any.tensor_copy(dst, vacc[r][:])

            # ---------------- phase 2 + 3, software-pipelined ----------------
            with (
                tc.tile_pool(name="pssc", bufs=4, space="PSUM") as pssc,
                tc.tile_pool(name="psot", bufs=2, space="PSUM") as psot,
                tc.tile_pool(name="pso", bufs=1, space="PSUM") as pso,
                tc.tile_pool(name="stgp", bufs=41) as stgp,
                tc.tile_pool(name="expp", bufs=11) as expp,
            ):
                def phase3_block(j):
                    """out-proj for q block j (consumes that block's gather)."""
                    qsl3 = slice(j * QB, (j + 1) * QB)
                    accs = [pso.tile([P, QB], f32, tag=f"po{occ}",
                                     name=f"po{occ}")
                            for occ in range(NCC)]
                    last = (j == NQB - 1)
                    order = (sorted(range(DCH), key=lambda m: (m % NCC, m // NCC))
                             if last else list(range(DCH)))
                    for i, mch in enumerate(order):
                        g_, cc_ = mch // NCC, mch % NCC
                        mt = ms.tile([P, QB], bf16, tag="mt", name="mt")
                        if last:
                            nc.sync.dma_start(mt[:], ag_out_h[j, cc_][g_, :, :])
                        else:
                            nc.sync.dma_start(mt[:], ag_out[j][g_, :, cc_, :])
                        for occ in range(NCC):
                            for wi, wt in enumerate((woh, wol)):
                                nc.tensor.matmul(
                                    accs[occ][:], wt[:, mch, occ * P:(occ + 1) * P],
                                    mt[:], start=(i == 0 and wi == 0),
                                    stop=(i == DCH - 1 and wi == 1))
                    for occ in range(NCC):
                        oo = op.tile([P, QB], f32, tag="oo", name="oo")
                        nc.any.tensor_copy(oo[:], accs[occ][:])
                        nc.sync.dma_start(outT[occ, :, qsl3], oo[:])

                def _gather(inp, outp):
                    nc.gpsimd.collective_compute(
                        "AllGather", mybir.AluOpType.bypass,
                        replica_groups=[[0, 1, 2, 3], [4, 5, 6, 7]],
                        ins=[inp], outs=[outp],
                    )

                def passA(qb, hp, h2):
                    hsl = slice(h2 * 64, (h2 + 1) * 64)
                    nkc = qb * KPB + KPB
                    rm = stat.tile([P, QB], bf16, tag="rm", name="rm")
                    nc.gpsimd.memset(rm[:], -3e38)
                    sts = []
                    for kc in range(nkc):
                        diag = kc - qb * KPB
                        off = max(0, diag) * P
                        psc = pssc.tile([P, QB], f32, tag="psc", name="psc")
                        ksl = slice(kc * P, (kc + 1) * P)
                        mvsl = slice(qb * QB + off, (qb + 1) * QB)
                        terms = ((KTh, QTh), (KTh, QTl), (KTl, QTh))
                        for ti, (kt, qt) in enumerate(terms):
                            nc.tensor.matmul(
                                psc[:, off:], kt[hsl, hp, ksl],
                                qt[hsl, hp, mvsl],
                                start=(ti == 0), stop=(ti == 2))
                        stg = stgp.tile([P, QB], f32, tag="stg", name="stg")
                        nc.scalar.activation(stg[:, off:], psc[:, off:], COPYF)
                        if diag >= 0:
                            nc.vector.tensor_tensor(
                                stg[:, off:off + P], stg[:, off:off + P],
                                maskT[:], mybir.AluOpType.add)
                        nc.vector.tensor_tensor(rm[:, off:], rm[:, off:],
                                                stg[:, off:], MAXOP)
                        sts.append((stg, off))
                    mrep = stat.tile([P, QB], bf16, tag="mrep", name="mrep")
                    nc.gpsimd.partition_all_reduce(
                        mrep[:], rm[:], P, bass_isa.ReduceOp.max)
                    return (qb, hp, h2, sts, mrep)

                def passB(st):
                    qb, hp, h2, sts, mrep = st
                    qsl = slice(qb * QB, (qb + 1) * QB)
                    hsl = slice(h2 * 64, (h2 + 1) * 64)
                    nkc = len(sts)
                    otp = psot.tile([VW, QB], f32, tag="otp", name="otp")
                    vg = slice((hp * 2 + h2) * VW, (hp * 2 + h2 + 1) * VW)
                    for kc, (stg, off) in enumerate(sts):
                        nc.vector.tensor_tensor(stg[:, off:], stg[:, off:],
                                                mrep[:, off:], SUB)
                        ex = expp.tile([P, QB], bf16, tag="ex", name="ex")
                        nc.scalar.activation(ex[:, off:], stg[:, off:], EXP,
                                             bias=mbias[:])
                        nc.tensor.matmul(otp[:, off:], Vsb[:, kc, vg],
                                         ex[:, off:],
                                         start=(kc == 0), stop=(kc == nkc - 1))
                    lnl = stat.tile([1, QB], f32, tag="lnl", name="lnl")
                    nc.scalar.activation(lnl[:], otp[64:65, :], LN)
                    rec = stat.tile([1, QB], f32, tag="rec",
                                    name="rec")
                    nc.scalar.activation(rec[:], lnl[:], EXP, scale=-1.0)
                    recb = stat.tile([64, QB], f32, tag="recb", name="recb")
                    nc.gpsimd.partition_broadcast(recb[:], rec[:], 64)
                    nc.vector.tensor_tensor(OT[hsl, hp, qsl], otp[0:64, :],
                                            recb[:], MULT)
                    # fire gathers / interleaved out-proj on block boundaries
                    if h2 == 1:
                        if qb == NQB - 1:
                            nc.sync.dma_start(ag_in_h[qb, hp][:],
                                              OT[:, hp, qsl])
                            _gather(ag_in_h[qb, hp][:], ag_out_h[qb, hp][:])
                        elif hp == NCC - 1:
                            nc.sync.dma_start(ag_in[qb][:], OT[:, :, qsl])
                            _gather(ag_in[qb][:], ag_out[qb][:])
                        if hp == NCC - 1 and qb >= 2:
                            phase3_block(qb - 2)

                blocks = [(qb, hp, h2) for qb in range(NQB)
                          for hp in range(NCC) for h2 in range(2)]
                pending = []
                for blk in blocks:
                    while len(pending) >= 2:
                        passB(pending.pop(0))
                    pending.append(passA(*blk))
                while pending:
                    passB(pending.pop(0))
                for j in range(max(0, NQB - 2), NQB):
                    phase3_block(j)

    nc.compile()
    return nc


_NC_CACHE = {}


def get_nc(**cfg):
    key = tuple(sorted(cfg.items()))
    if key not in _NC_CACHE:
        _NC_CACHE[key] = build_nc(**cfg)
    return _NC_CACHE[key]


def _col_index(g):
    p = np.arange(CPC)
    return (p % HD) * HEADS + (HPC * g + p // HD)


def _ow_row_index():
    r = np.arange(D)
    m, p128 = r // P, r % P
    g_, cc = m // NCC, m % NCC
    p256 = cc * P + p128
    lh, hd = p256 // HD, p256 % HD
    return hd * HEADS + (HPC * g_ + lh)


def make_in_maps(x, qw, kw, vw, ow, s=S):
    scale = 1.0 / np.sqrt(np.float32(D))
    qws = (qw * scale).astype(np.float32)
    ow_perm = np.ascontiguousarray(ow[_ow_row_index()])
    in_maps = []
    xTs = [np.ascontiguousarray(x[b, :s].T) for b in range(B)]
    for c in range(N_CORES):
        b, g = c // GROUPS, c % GROUPS
        cols = _col_index(g)
        in_maps.append({
            "xT": xTs[b],
            "wq": np.ascontiguousarray(qws[:, cols]),
            "wk": np.ascontiguousarray(kw[:, cols]),
            "wv": np.ascontiguousarray(vw[:, cols]),
            "wo": np.ascontiguousarray(ow_perm[:, g * CPC:(g + 1) * CPC]),
        })
    return in_maps


def assemble_output(results, s=S):
    out = np.empty((B, s, D), dtype=np.float32)
    for c in range(N_CORES):
        b, g = c // GROUPS, c % GROUPS
        oT = results[c]["outT"]  # [NCC, P, s]
        for occ in range(NCC):
            out[b, :, g * CPC + occ * P:(g * CPC + (occ + 1) * P)] = oT[occ].T
    return out


def run_on_hw(x, qw, kw, vw, ow, trace=False, **cfg_over):
    cfg = dict(DEFAULT_CFG)
    cfg.update(cfg_over)
    s = cfg["s"]
    nc = get_nc(**cfg)
    in_maps = make_in_maps(x, qw, kw, vw, ow, s=s)
    res = run_bass_kernel_spmd(nc, in_maps, core_ids=list(range(N_CORES)),
                               trace=trace)
    return assemble_output(res.results, s=s), res


def kernel(x, qw, kw, vw, ow):
    out, _ = run_on_hw(np.asarray(x, dtype=np.float32),
                       np.asarray(qw, dtype=np.float32),
                       np.asarray(kw, dtype=np.float32),
                       np.asarray(vw, dtype=np.float32),
                       np.asarray(ow, dtype=np.float32))
    return out



# revision 11
# speedup vs baseline: 1.1923x; 1.1923x over previous
"""Multi-head attention (dense_transformer) on 8 TRN2 NeuronCores.

Sharding: 2-way data parallel over batch x 4-way tensor parallel over heads.
Core c handles batch b=c//4 and heads {4g..4g+3} where g=c%4 (4 heads, 256
channels per core; channels of head h are qw columns {hd*16+h}).

Architecture v3 ("fp16 + fp8-DoubleRow pseudo-fp32"):
  The pseudo-fp32 matmuls (Q/K projections and K^T Q scores) need ~15-bit
  operand mantissas because score std is ~256 and softmax is near-argmax.
  v2 used 3-term bf16 splits (3 passes).  v3 uses:
    main term:  fp16 x fp16 (11-bit mantissas), 1.0 PE cycles/column
    both cross terms: one fp8-e4m3 matmul in DoubleRow perf mode (two
      stationary/moving stream pairs summed into one PSUM output) at 0.5
      cycles/column.
  => 1.5 pass-equivalents instead of 3.  Per-stream power-of-2 scales keep
  every operand inside e4m3/fp16 range; all passes of one accumulation
  produce the same product scale, which is folded into the EXP scale.

  Score-side operand scales (score PSUM = 128*score; every e4m3 operand
  stays under 224 so the e4m3/e4m3fn variant ambiguity is moot):
    KH16=fp16(4K), KL8=e4m3(4(K-Kh)), KH8=e4m3(Kh)
    QH16=fp16(32Q), QL8=e4m3(128(Q-Qh)), QH8=e4m3(32Qh)
    DR streams: (KL8,QH8) + (KH8,QL8) -> 128*(Kl Qh + Kh Ql)
  Projection pass scales: QPSUM=32768*Q (wq folded 1/32), KPSUM=1024*K;
  both drain with the same ActCopy scale 1/512.

  x and weight hi/lo splits are precomputed on the HOST (no on-device
  split work; x ships as fp16 + e4m3 pair, same bytes as one f32 copy).

  Causal mask is applied on the PE: an extra rank-structured matmul term
  (stationary -1e10*I bf16, moving strictly-lower-triangular ones) adds
  -1e10 to masked elements of the diagonal chunks inside PSUM, before any
  drain -- no DVE masking.

  V projection: single fp16 pass (xh * fp16(wv/16)); V, exp weights, OT,
  and the out-projection all run in fp16 (11-bit) instead of bf16, which
  *reduces* error vs v2 while the out-projection drops to a single pass.

  Softmax sum reciprocal: vector-engine reciprocal_approx_fast (1 DVE op)
  instead of Ln+Exp on ACT (kills the activation-table thrash).

  Engine balance per score chunk: PSUM drain (ACT Copy), running max
  (DVE), subtract (DVE), EXP (ACT).
"""
import sys

sys.path.insert(0, "/opt/trn_rl_repo")

import numpy as np
import ml_dtypes

import concourse.bass as bass
import concourse.mybir as mybir
import concourse.tile as tile
from concourse import bacc
from concourse import bass_isa
from concourse.bass_utils import run_bass_kernel_spmd

# ---- problem constants (hardcoded per harness contract) ----
B, S, D, HEADS = 2, 2048, 1024, 16
N_CORES = 8
GROUPS = 4                 # head-groups == cores per batch
HPC = HEADS // GROUPS      # heads per core (4)
HD = D // HEADS            # 64
CPC = HPC * HD             # channels per core (256)
P = 128
NCC = CPC // P             # col chunks per core (2)
DCH = D // P               # contraction chunks (8)
QB = 512                   # q block width (1 PSUM bank of f32)

f32 = mybir.dt.float32
bf16 = mybir.dt.bfloat16
fp16 = mybir.dt.float16
e4m3 = mybir.dt.float8e4

AX = mybir.AxisListType
EXP = mybir.ActivationFunctionType.Exp
MAXOP = mybir.AluOpType.max
SUB = mybir.AluOpType.subtract
MULT = mybir.AluOpType.mult
COPYF = mybir.ActivationFunctionType.Copy
DR = mybir.MatmulPerfMode.DoubleRow

DEFAULT_CFG = dict(s=S)

# host-side split scales
SWQ = 2048.0   # wq-tilde (=qw/32) fp16-hi scale
SWK = 64.0     # kw fp16-hi scale
SX = 16.0      # x fp16-hi scale
SXL = 256.0    # x e4m3-lo scale
DRAINQ = 1.0 / 1024.0  # QPSUM(32768Q) -> fp16(32Q)
DRAINK = 1.0 / 256.0   # KPSUM(1024K) -> fp16(4K)
SSCORE = 128.0         # score PSUM scale


def build_nc(s=S, dbg=False):
    assert s % QB == 0
    NQB = s // QB            # 512-wide q blocks
    NKC = s // P             # 128-wide k chunks
    KPB = QB // P            # k chunks per q block on the diagonal (4)
    VW = 65                  # V channels per (hp,h2) incl the ones column
    NH2 = NCC * 2            # head slots per core (4)

    nc = bacc.Bacc("TRN2", target_bir_lowering=False, debug=False,
                   num_devices=N_CORES)
    xh16d = nc.dram_tensor("xh16", [D, s], fp16, kind="ExternalInput").ap()
    xdrd = nc.dram_tensor("xdr", [D, 2, s], e4m3, kind="ExternalInput").ap()
    wqh = nc.dram_tensor("wqh", [D, CPC], fp16, kind="ExternalInput").ap()
    wqdr = nc.dram_tensor("wqdr", [D, 2, CPC], e4m3, kind="ExternalInput").ap()
    wkh = nc.dram_tensor("wkh", [D, CPC], fp16, kind="ExternalInput").ap()
    wkdr = nc.dram_tensor("wkdr", [D, 2, CPC], e4m3, kind="ExternalInput").ap()
    wvh = nc.dram_tensor("wvh", [D, CPC], fp16, kind="ExternalInput").ap()
    woh = nc.dram_tensor("woh", [D, CPC], fp16, kind="ExternalInput").ap()
    mskst = nc.dram_tensor("mskst", [P, P], bf16, kind="ExternalInput").ap()
    mskmv = nc.dram_tensor("mskmv", [P, P], bf16, kind="ExternalInput").ap()
    outT = nc.dram_tensor("outT", [NCC, P, s], f32, kind="ExternalOutput").ap()

    with tile.TileContext(nc) as tc:
        with (
            tc.tile_pool(name="cpool", bufs=1) as cpool,
            tc.tile_pool(name="wpool", bufs=1) as wpool,
            tc.tile_pool(name="big", bufs=1) as big,
            tc.tile_pool(name="stat", bufs=3) as stat,
            tc.tile_pool(name="ms", bufs=6) as ms,
            tc.tile_pool(name="op", bufs=2) as op,
            tc.tile_pool(name="dram", bufs=1, space="DRAM") as dpool,
        ):
            # per-(qb,hp) gather buffers; last qb additionally split by h2
            ag_in = {}
            ag_out = {}
            for qb in range(NQB):
                for hp in range(NCC):
                    if qb == NQB - 1:
                        for h2 in range(2):
                            ag_in[qb, hp, h2] = dpool.tile(
                                [64, QB], fp16, tag=f"agi{qb}_{hp}_{h2}",
                                name=f"agi{qb}_{hp}_{h2}")
                            ag_out[qb, hp, h2] = dpool.tile(
                                [GROUPS, 64, QB], fp16,
                                tag=f"ago{qb}_{hp}_{h2}",
                                name=f"ago{qb}_{hp}_{h2}")
                    else:
                        ag_in[qb, hp] = dpool.tile(
                            [P, QB], fp16, tag=f"agi{qb}_{hp}",
                            name=f"agi{qb}_{hp}")
                        ag_out[qb, hp] = dpool.tile(
                            [GROUPS, P, QB], fp16, tag=f"ago{qb}_{hp}",
                            name=f"ago{qb}_{hp}")

            # weights + constants in SBUF
            msk_st = cpool.tile([P, P], bf16, tag="mskst")
            msk_mv = cpool.tile([P, P], bf16, tag="mskmv")
            wqh_sb = wpool.tile([P, DCH, CPC], fp16, tag="wqh")
            wqdr_sb = wpool.tile([P, DCH, 2, CPC], e4m3, tag="wqdr")
            wkh_sb = wpool.tile([P, DCH, CPC], fp16, tag="wkh")
            wkdr_sb = wpool.tile([P, DCH, 2, CPC], e4m3, tag="wkdr")
            wvh_sb = wpool.tile([P, DCH, CPC], fp16, tag="wvh")
            woh_sb = wpool.tile([P, DCH, CPC], fp16, tag="woh")
            for di in range(DCH):
                dsl = slice(di * P, (di + 1) * P)
                nc.sync.dma_start(wqh_sb[:, di, :], wqh[dsl, :])
                nc.sync.dma_start(wqdr_sb[:, di], wqdr[dsl])
                nc.sync.dma_start(wkh_sb[:, di, :], wkh[dsl, :])
                nc.sync.dma_start(wkdr_sb[:, di], wkdr[dsl])
                nc.sync.dma_start(wvh_sb[:, di, :], wvh[dsl, :])
                nc.sync.dma_start(woh_sb[:, di, :], woh[dsl, :])
            nc.sync.dma_start(msk_st[:], mskst)
            nc.sync.dma_start(msk_mv[:], mskmv)

            QH16 = big.tile([P, NCC, s], fp16, tag="QH16")
            QDR = big.tile([P, NCC, 2, s], e4m3, tag="QDR")
            KH16 = big.tile([P, NCC, s], fp16, tag="KH16")
            KDR = big.tile([P, NCC, 2, s], e4m3, tag="KDR")
            # Vhat[k, :]: 4 groups of 65 cols: 64 V channels + a ones col
            Vsb = big.tile([P, NKC, NH2 * VW], fp16, tag="Vsb")
            OT = big.tile([P, NCC, s], fp16, tag="OT")

            for g in range(NH2):
                nc.gpsimd.memset(Vsb[:, :, g * VW + 64], 1.0)
            mbias = cpool.tile([P, 1], f32, tag="mbias")
            nc.gpsimd.memset(mbias[:], -6.0)

            # ---------------- phase 1: projections ----------------
            with (
                tc.tile_pool(name="psq", bufs=1, space="PSUM") as psq,
                tc.tile_pool(name="psv", bufs=1, space="PSUM") as psv,
                tc.tile_pool(name="xs", bufs=2) as xs,
                tc.tile_pool(name="xtmp", bufs=2) as xtmp,
            ):
                for qb in range(NQB):
                    qsl = slice(qb * QB, (qb + 1) * QB)
                    xh = []
                    xdr = []
                    for di in range(DCH):
                        dsl = slice(di * P, (di + 1) * P)
                        xht = xs.tile([P, QB], fp16, tag=f"xh{di}",
                                      name=f"xh{di}")
                        nc.sync.dma_start(xht[:], xh16d[dsl, qsl])
                        xdt = xs.tile([P, 2, QB], e4m3, tag=f"xd{di}",
                                      name=f"xd{di}")
                        nc.sync.dma_start(xdt[:], xdrd[dsl, :, qsl])
                        xh.append(xht)
                        xdr.append(xdt)
                    for nm, whs, wds, hi_t, dr_t in (
                            ("q", wqh_sb, wqdr_sb, QH16, QDR),
                            ("k", wkh_sb, wkdr_sb, KH16, KDR)):
                        dscale = DRAINQ if nm == "q" else DRAINK
                        for cc in range(NCC):
                            csl = slice(cc * P, (cc + 1) * P)
                            acc = psq.tile([P, QB], f32, tag=f"a{nm}{cc}",
                                           name=f"a{nm}{cc}")
                            for di in range(DCH):
                                nc.tensor.matmul(
                                    acc[:], whs[:, di, csl], xh[di][:],
                                    start=(di == 0), stop=False)
                                nc.tensor.matmul(
                                    acc[:], wds[:, di, :, csl], xdr[di][:],
                                    start=False, stop=(di == DCH - 1),
                                    perf_mode=DR)
                            nc.scalar.activation(hi_t[:, cc, qsl], acc[:],
                                                 COPYF, scale=dscale)
                            if nm == "q":
                                # QL8 = e4m3(128 Ql): STT gives 32Ql in f32,
                                # then ACT rescales x4 into e4m3
                                t32 = xtmp.tile([P, QB], f32, tag="t32",
                                                name="t32")
                                nc.vector.scalar_tensor_tensor(
                                    t32[:], acc[:], dscale,
                                    hi_t[:, cc, qsl], MULT, SUB)
                                nc.scalar.activation(
                                    dr_t[:, cc, 1, qsl], t32[:], COPYF,
                                    scale=4.0)
                                # QH8 = e4m3(32 Qh): plain cast of QH16
                                nc.vector.tensor_copy(
                                    dr_t[:, cc, 0, qsl], hi_t[:, cc, qsl])
                            else:
                                # KL8 = e4m3(4 Kl)
                                nc.vector.scalar_tensor_tensor(
                                    dr_t[:, cc, 0, qsl], acc[:], dscale,
                                    hi_t[:, cc, qsl], MULT, SUB)
                                # KH8 = e4m3(Kh) = KH16 x 1/4
                                nc.scalar.activation(
                                    dr_t[:, cc, 1, qsl], hi_t[:, cc, qsl],
                                    COPYF, scale=0.25)
                    for r in range(KPB):
                        vacc = psv.tile([P, CPC], f32, tag=f"pv{r}",
                                        name=f"pv{r}")
                        for di in range(DCH):
                            nc.tensor.matmul(
                                vacc[:], xh[di][:, r * P:(r + 1) * P],
                                wvh_sb[:, di, :],
                                start=(di == 0), stop=(di == DCH - 1))
                        ki = qb * KPB + r
                        dst = Vsb[:, ki].rearrange(
                            "p (g w) -> p g w", w=VW)[:, :, 0:64]
                        src = vacc[:].rearrange("p (g w) -> p g w", w=64)
                        nc.scalar.activation(dst, src, COPYF)

            # ---------------- phase 2 + 3, software-pipelined ----------------
            with (
                tc.tile_pool(name="pssc", bufs=4, space="PSUM") as pssc,
                tc.tile_pool(name="psot", bufs=2, space="PSUM") as psot,
                tc.tile_pool(name="pso", bufs=1, space="PSUM") as pso,
                tc.tile_pool(name="stgp", bufs=34) as stgp,
                tc.tile_pool(name="expp", bufs=8) as expp,
            ):
                def phase3_block(j):
                    """out-proj for q block j (consumes that block's gather)."""
                    qsl3 = slice(j * QB, (j + 1) * QB)
                    accs = [pso.tile([P, QB], f32, tag=f"po{occ}",
                                     name=f"po{occ}")
                            for occ in range(NCC)]
                    last = (j == NQB - 1)
                    # cc-major order so the last block's late gathers (hp=1)
                    # are needed as late as possible
                    order = (sorted(range(DCH), key=lambda m: (m % NCC, m // NCC))
                             if last else list(range(DCH)))
                    for i, mch in enumerate(order):
                        g_, cc_ = mch // NCC, mch % NCC
                        mt = ms.tile([P, QB], fp16, tag="mt", name="mt")
                        if last:
                            nc.sync.dma_start(mt[0:64, :],
                                              ag_out[j, cc_, 0][g_])
                            nc.sync.dma_start(mt[64:128, :],
                                              ag_out[j, cc_, 1][g_])
                        else:
                            nc.sync.dma_start(mt[:], ag_out[j, cc_][g_])
                        for occ in range(NCC):
                            nc.tensor.matmul(
                                accs[occ][:],
                                woh_sb[:, mch, occ * P:(occ + 1) * P],
                                mt[:], start=(i == 0), stop=(i == DCH - 1))
                    for occ in range(NCC):
                        oo = op.tile([P, QB], f32, tag="oo", name="oo")
                        nc.any.tensor_copy(oo[:], accs[occ][:])
                        nc.sync.dma_start(outT[occ, :, qsl3], oo[:])

                def _gather(inp, outp):
                    nc.gpsimd.collective_compute(
                        "AllGather", mybir.AluOpType.bypass,
                        replica_groups=[[0, 1, 2, 3], [4, 5, 6, 7]],
                        ins=[inp], outs=[outp],
                    )

                def passA(qb, hp, h2):
                    hsl = slice(h2 * 64, (h2 + 1) * 64)
                    nkc = qb * KPB + KPB
                    rm = stat.tile([P, QB], bf16, tag="rm", name="rm")
                    nc.gpsimd.memset(rm[:], -3e38)
                    sts = []
                    for kc in range(nkc):
                        diag = kc - qb * KPB
                        off = max(0, diag) * P
                        psc = pssc.tile([P, QB], f32, tag="psc", name="psc")
                        ksl = slice(kc * P, (kc + 1) * P)
                        mvsl = slice(qb * QB + off, (qb + 1) * QB)
                        nc.tensor.matmul(
                            psc[:, off:], KH16[hsl, hp, ksl],
                            QH16[hsl, hp, mvsl], start=True, stop=False)
                        nc.tensor.matmul(
                            psc[:, off:], KDR[hsl, hp, :, ksl],
                            QDR[hsl, hp, :, mvsl], start=False,
                            stop=(diag < 0), perf_mode=DR)
                        if diag >= 0:
                            nc.tensor.matmul(
                                psc[:, off:off + P], msk_st[:],
                                msk_mv[:], start=False, stop=True)
                        stg = stgp.tile([P, QB], f32, tag="stg", name="stg")
                        nc.scalar.activation(stg[:, off:], psc[:, off:], COPYF)
                        nc.vector.tensor_tensor(rm[:, off:], rm[:, off:],
                                                stg[:, off:], MAXOP)
                        sts.append((stg, off))
                    mrep = stat.tile([P, QB], bf16, tag="mrep", name="mrep")
                    nc.gpsimd.partition_all_reduce(
                        mrep[:], rm[:], P, bass_isa.ReduceOp.max)
                    return (qb, hp, h2, sts, mrep)

                def passB(st):
                    qb, hp, h2, sts, mrep = st
                    qsl = slice(qb * QB, (qb + 1) * QB)
                    hsl = slice(h2 * 64, (h2 + 1) * 64)
                    nkc = len(sts)
                    otp = psot.tile([VW, QB], f32, tag="otp", name="otp")
                    vg = slice((hp * 2 + h2) * VW, (hp * 2 + h2 + 1) * VW)
                    for kc, (stg, off) in enumerate(sts):
                        nc.vector.tensor_tensor(stg[:, off:], stg[:, off:],
                                                mrep[:, off:], SUB)
                        ex = expp.tile([P, QB], fp16, tag="ex", name="ex")
                        nc.scalar.activation(ex[:, off:], stg[:, off:], EXP,
                                             bias=mbias[:], scale=1.0 / SSCORE)
                        nc.tensor.matmul(otp[:, off:], Vsb[:, kc, vg],
                                         ex[:, off:],
                                         start=(kc == 0), stop=(kc == nkc - 1))
                    lsb = stat.tile([1, QB], f32, tag="lsb", name="lsb")
                    nc.vector.tensor_copy(lsb[:], otp[64:65, :])
                    rec = stat.tile([1, QB], f32, tag="rec", name="rec")
                    nc.vector.reciprocal_approx_fast(rec[:], lsb[:])
                    recb = stat.tile([64, QB], f32, tag="recb", name="recb")
                    nc.gpsimd.partition_broadcast(recb[:], rec[:], 64)
                    nc.vector.tensor_tensor(OT[hsl, hp, qsl], otp[0:64, :],
                                            recb[:], MULT)
                    # fire gathers / interleaved out-proj on block boundaries
                    if qb == NQB - 1:
                        nc.sync.dma_start(ag_in[qb, hp, h2][:],
                                          OT[hsl, hp, qsl])
                        _gather(ag_in[qb, hp, h2][:], ag_out[qb, hp, h2][:])
                        if hp == NCC - 1 and h2 == 1:
                            pass  # tail outproj issued after the loop
                    elif h2 == 1:
                        nc.sync.dma_start(ag_in[qb, hp][:], OT[:, hp, qsl])
                        _gather(ag_in[qb, hp][:], ag_out[qb, hp][:])
                        if hp == NCC - 1 and qb >= 1:
                            phase3_block(qb - 1)

                blocks = [(qb, hp, h2) for qb in range(NQB)
                          for hp in range(NCC) for h2 in range(2)]
                pending = []
                for blk in blocks:
                    while len(pending) >= 2:
                        passB(pending.pop(0))
                    pending.append(passA(*blk))
                while pending:
                    passB(pending.pop(0))
                for j in range(max(0, NQB - 2), NQB):
                    phase3_block(j)

    nc.compile()
    return nc


_NC_CACHE = {}


def get_nc(**cfg):
    key = tuple(sorted(cfg.items()))
    if key not in _NC_CACHE:
        _NC_CACHE[key] = build_nc(**cfg)
    return _NC_CACHE[key]


def _col_index(g):
    p = np.arange(CPC)
    return (p % HD) * HEADS + (HPC * g + p // HD)


def _ow_row_index():
    r = np.arange(D)
    m, p128 = r // P, r % P
    g_, cc = m // NCC, m % NCC
    p256 = cc * P + p128
    lh, hd = p256 // HD, p256 % HD
    return hd * HEADS + (HPC * g_ + lh)


def _split16(w, shi):
    """w -> (fp16(shi*w), e4m3(shi*(w-hi/shi)), e4m3(shi/16*whi))"""
    hi = (shi * w).astype(np.float16)
    lo = shi * w - hi.astype(np.float32)
    l8 = lo.astype(ml_dtypes.float8_e4m3)
    h8 = (hi.astype(np.float32) / 16.0).astype(ml_dtypes.float8_e4m3)
    return hi, l8, h8


def make_in_maps(x, qw, kw, vw, ow, s=S):
    scale = 1.0 / np.sqrt(np.float32(D))
    qws = (qw * scale).astype(np.float32)
    ow_perm = np.ascontiguousarray(ow[_ow_row_index()])

    # x hi/lo splits, shared per batch
    xsplits = []
    for b in range(B):
        xT = np.ascontiguousarray(x[b, :s].T).astype(np.float32)
        xh = (SX * xT).astype(np.float16)          # fp16(16 x)
        xl = SX * xT - xh.astype(np.float32)       # 16 xl
        xdr = np.empty((D, 2, s), dtype=ml_dtypes.float8_e4m3)
        xdr[:, 0, :] = xh.astype(ml_dtypes.float8_e4m3)   # e4m3(16 xh)
        xdr[:, 1, :] = (SXL / SX * xl).astype(ml_dtypes.float8_e4m3)
        xsplits.append((xh, xdr))

    mskst = (-1e10 * np.eye(P, dtype=np.float32)).astype(ml_dtypes.bfloat16)
    mskmv = np.tril(np.ones((P, P), dtype=np.float32), -1).astype(
        ml_dtypes.bfloat16)

    in_maps = []
    for c in range(N_CORES):
        b, g = c // GROUPS, c % GROUPS
        cols = _col_index(g)
        wq = np.ascontiguousarray(qws[:, cols])
        wk = np.ascontiguousarray(kw[:, cols]).astype(np.float32)
        qh, ql8, qh8 = _split16(wq, SWQ)
        kh, kl8, kh8 = _split16(wk, SWK)
        wqdr = np.empty((D, 2, CPC), dtype=ml_dtypes.float8_e4m3)
        wqdr[:, 0, :] = ql8
        wqdr[:, 1, :] = qh8
        wkdr = np.empty((D, 2, CPC), dtype=ml_dtypes.float8_e4m3)
        wkdr[:, 0, :] = kl8
        wkdr[:, 1, :] = kh8
        xh, xdr = xsplits[b]
        in_maps.append({
            "xh16": xh,
            "xdr": xdr,
            "wqh": qh,
            "wqdr": wqdr,
            "wkh": kh,
            "wkdr": wkdr,
            "wvh": (np.ascontiguousarray(vw[:, cols]) / SX).astype(
                np.float16),
            "woh": np.ascontiguousarray(
                ow_perm[:, g * CPC:(g + 1) * CPC]).astype(np.float16),
            "mskst": mskst,
            "mskmv": mskmv,
        })
    return in_maps


def assemble_output(results, s=S):
    out = np.empty((B, s, D), dtype=np.float32)
    for c in range(N_CORES):
        b, g = c // GROUPS, c % GROUPS
        oT = results[c]["outT"]  # [NCC, P, s]
        for occ in range(NCC):
            out[b, :, g * CPC + occ * P:(g * CPC + (occ + 1) * P)] = oT[occ].T
    return out


def run_on_hw(x, qw, kw, vw, ow, trace=False, **cfg_over):
    cfg = dict(DEFAULT_CFG)
    cfg.update(cfg_over)
    s = cfg["s"]
    nc = get_nc(**cfg)
    in_maps = make_in_maps(x, qw, kw, vw, ow, s=s)
    res = run_bass_kernel_spmd(nc, in_maps, core_ids=list(range(N_CORES)),
                               trace=trace)
    return assemble_output(res.results, s=s), res


def kernel(x, qw, kw, vw, ow):
    out, _ = run_on_hw(np.asarray(x, dtype=np.float32),
                       np.asarray(qw, dtype=np.float32),
                       np.asarray(kw, dtype=np.float32),
                       np.asarray(vw, dtype=np.float32),
                       np.asarray(ow, dtype=np.float32))
    return out


# revision 16
# speedup vs baseline: 1.2340x; 1.0350x over previous
"""Multi-head attention (dense_transformer) on 8 TRN2 NeuronCores.

Sharding: 2-way data parallel over batch x 4-way tensor parallel over heads.
Core c handles batch b=c//4 and heads {4g..4g+3} where g=c%4 (4 heads, 256
channels per core; channels of head h are qw columns {hd*16+h}).

Architecture v3 ("fp16 + fp8-DoubleRow pseudo-fp32"):
  The pseudo-fp32 matmuls (Q/K projections and K^T Q scores) need ~15-bit
  operand mantissas because score std is ~256 and softmax is near-argmax.
  v2 used 3-term bf16 splits (3 passes).  v3 uses:
    main term:  fp16 x fp16 (11-bit mantissas), 1.0 PE cycles/column
    both cross terms: one fp8-e4m3 matmul in DoubleRow perf mode (two
      stationary/moving stream pairs summed into one PSUM output) at 0.5
      cycles/column.
  => 1.5 pass-equivalents instead of 3.  Per-stream power-of-2 scales keep
  every operand inside e4m3/fp16 range; all passes of one accumulation
  produce the same product scale, which is folded into the EXP scale.

  Score-side operand scales (score PSUM = 128*score; every e4m3 operand
  stays under 224 so the e4m3/e4m3fn variant ambiguity is moot):
    KH16=fp16(4K), KL8=e4m3(4(K-Kh)), KH8=e4m3(Kh)
    QH16=fp16(32Q), QL8=e4m3(128(Q-Qh)), QH8=e4m3(32Qh)
    DR streams: (KL8,QH8) + (KH8,QL8) -> 128*(Kl Qh + Kh Ql)
  Projection pass scales: QPSUM=32768*Q (wq folded 1/32), KPSUM=1024*K;
  both drain with the same ActCopy scale 1/512.

  x and weight hi/lo splits are precomputed on the HOST (no on-device
  split work; x ships as fp16 + e4m3 pair, same bytes as one f32 copy).

  Causal mask is applied on the PE: an extra rank-structured matmul term
  (stationary -1e10*I bf16, moving strictly-lower-triangular ones) adds
  -1e10 to masked elements of the diagonal chunks inside PSUM, before any
  drain -- no DVE masking.

  V projection: single fp16 pass (xh * fp16(wv/16)); V, exp weights, OT,
  and the out-projection all run in fp16 (11-bit) instead of bf16, which
  *reduces* error vs v2 while the out-projection drops to a single pass.

  Softmax sum reciprocal: vector-engine reciprocal_approx_fast (1 DVE op)
  instead of Ln+Exp on ACT (kills the activation-table thrash).

  Engine balance per score chunk: PSUM drain (ACT Copy), running max
  (DVE), subtract (DVE), EXP (ACT).
"""
import sys

sys.path.insert(0, "/opt/trn_rl_repo")

import numpy as np
import ml_dtypes

import concourse.bass as bass
import concourse.mybir as mybir
import concourse.tile as tile
from concourse import bacc
from concourse import bass_isa
from concourse.bass_utils import run_bass_kernel_spmd

# ---- problem constants (hardcoded per harness contract) ----
B, S, D, HEADS = 2, 2048, 1024, 16
N_CORES = 8
GROUPS = 4                 # head-groups == cores per batch
HPC = HEADS // GROUPS      # heads per core (4)
HD = D // HEADS            # 64
CPC = HPC * HD             # channels per core (256)
P = 128
NCC = CPC // P             # col chunks per core (2)
DCH = D // P               # contraction chunks (8)
QB = 512                   # q block width (1 PSUM bank of f32)

f32 = mybir.dt.float32
bf16 = mybir.dt.bfloat16
fp16 = mybir.dt.float16
e4m3 = mybir.dt.float8e4

AX = mybir.AxisListType
EXP = mybir.ActivationFunctionType.Exp
MAXOP = mybir.AluOpType.max
SUB = mybir.AluOpType.subtract
MULT = mybir.AluOpType.mult
COPYF = mybir.ActivationFunctionType.Copy
DR = mybir.MatmulPerfMode.DoubleRow

DEFAULT_CFG = dict(s=S)

# host-side split scales
SWQ = 2048.0   # wq-tilde (=qw/32) fp16-hi scale
SWK = 64.0     # kw fp16-hi scale
SX = 16.0      # x fp16-hi scale
SXL = 256.0    # x e4m3-lo scale
DRAINQ = 1.0 / 1024.0  # QPSUM(32768Q) -> fp16(32Q)
DRAINK = 1.0 / 256.0   # KPSUM(1024K) -> fp16(4K)
SSCORE = 128.0         # score PSUM scale


def build_nc(s=S, dbg=False):
    assert s % QB == 0
    NQB = s // QB            # 512-wide q blocks
    NKC = s // P             # 128-wide k chunks
    KPB = QB // P            # k chunks per q block on the diagonal (4)
    VW = 65                  # V channels per (hp,h2) incl the ones column
    NH2 = NCC * 2            # head slots per core (4)

    nc = bacc.Bacc("TRN2", target_bir_lowering=False, debug=False,
                   num_devices=N_CORES)
    xh16d = nc.dram_tensor("xh16", [D, s], fp16, kind="ExternalInput").ap()
    xdrd = nc.dram_tensor("xdr", [D, 2, s], e4m3, kind="ExternalInput").ap()
    wqh = nc.dram_tensor("wqh", [D, CPC], fp16, kind="ExternalInput").ap()
    wqdr = nc.dram_tensor("wqdr", [D, 2, CPC], e4m3, kind="ExternalInput").ap()
    wkh = nc.dram_tensor("wkh", [D, CPC], fp16, kind="ExternalInput").ap()
    wkdr = nc.dram_tensor("wkdr", [D, 2, CPC], e4m3, kind="ExternalInput").ap()
    wvh = nc.dram_tensor("wvh", [D, CPC], fp16, kind="ExternalInput").ap()
    woh = nc.dram_tensor("woh", [D, CPC], fp16, kind="ExternalInput").ap()
    mskst = nc.dram_tensor("mskst", [P, P], bf16, kind="ExternalInput").ap()
    mskmv = nc.dram_tensor("mskmv", [P, P], bf16, kind="ExternalInput").ap()
    outT = nc.dram_tensor("outT", [NCC, P, s], f32, kind="ExternalOutput").ap()

    with tile.TileContext(nc) as tc:
        with (
            tc.tile_pool(name="cpool", bufs=1) as cpool,
            tc.tile_pool(name="wpool", bufs=1) as wpool,
            tc.tile_pool(name="big", bufs=1) as big,
            tc.tile_pool(name="stat", bufs=3) as stat,
            tc.tile_pool(name="ms", bufs=6) as ms,
            tc.tile_pool(name="op", bufs=2) as op,
            tc.tile_pool(name="dram", bufs=1, space="DRAM") as dpool,
        ):
            # per-(qb,hp) gather buffers; last qb additionally split by h2
            ag_in = {}
            ag_out = {}
            for qb in range(NQB):
                for hp in range(NCC):
                    if qb == NQB - 1:
                        for h2 in range(2):
                            ag_in[qb, hp, h2] = dpool.tile(
                                [64, QB], fp16, tag=f"agi{qb}_{hp}_{h2}",
                                name=f"agi{qb}_{hp}_{h2}")
                            ag_out[qb, hp, h2] = dpool.tile(
                                [GROUPS, 64, QB], fp16,
                                tag=f"ago{qb}_{hp}_{h2}",
                                name=f"ago{qb}_{hp}_{h2}")
                    else:
                        ag_in[qb, hp] = dpool.tile(
                            [P, QB], fp16, tag=f"agi{qb}_{hp}",
                            name=f"agi{qb}_{hp}")
                        ag_out[qb, hp] = dpool.tile(
                            [GROUPS, P, QB], fp16, tag=f"ago{qb}_{hp}",
                            name=f"ago{qb}_{hp}")

            # weights + constants in SBUF
            msk_st = cpool.tile([P, P], bf16, tag="mskst")
            msk_mv = cpool.tile([P, P], bf16, tag="mskmv")
            wqh_sb = wpool.tile([P, DCH, CPC], fp16, tag="wqh")
            wqdr_sb = wpool.tile([P, DCH, 2, CPC], e4m3, tag="wqdr")
            wkh_sb = wpool.tile([P, DCH, CPC], fp16, tag="wkh")
            wkdr_sb = wpool.tile([P, DCH, 2, CPC], e4m3, tag="wkdr")
            wvh_sb = wpool.tile([P, DCH, CPC], fp16, tag="wvh")
            woh_sb = wpool.tile([P, DCH, CPC], fp16, tag="woh")
            # one DMA per weight tensor ([D,...] viewed as [P, DCH, ...]);
            # wo deferred until after qb0's x tiles (not needed in phase 1)
            nc.sync.dma_start(
                wqh_sb[:], wqh.rearrange("(dc p) c -> p dc c", p=P))
            nc.sync.dma_start(
                wqdr_sb[:], wqdr.rearrange("(dc p) t c -> p dc t c", p=P))
            nc.sync.dma_start(
                wkh_sb[:], wkh.rearrange("(dc p) c -> p dc c", p=P))
            nc.sync.dma_start(
                wkdr_sb[:], wkdr.rearrange("(dc p) t c -> p dc t c", p=P))
            nc.sync.dma_start(msk_st[:], mskst)
            nc.sync.dma_start(msk_mv[:], mskmv)
            nc.sync.dma_start(
                wvh_sb[:], wvh.rearrange("(dc p) c -> p dc c", p=P))

            QH16 = big.tile([P, NCC, s], fp16, tag="QH16")
            QDR = big.tile([P, NCC, 2, s], e4m3, tag="QDR")
            KH16 = big.tile([P, NCC, s], fp16, tag="KH16")
            KDR = big.tile([P, NCC, 2, s], e4m3, tag="KDR")
            # Vhat[k, :]: 4 groups of 65 cols: 64 V channels + a ones col
            Vsb = big.tile([P, NKC, NH2 * VW], fp16, tag="Vsb")
            OT = big.tile([P, NCC, s], fp16, tag="OT")

            for g in range(NH2):
                nc.gpsimd.memset(Vsb[:, :, g * VW + 64], 1.0)
            mbias = cpool.tile([P, 1], f32, tag="mbias")
            nc.gpsimd.memset(mbias[:], -6.0)

            # ---------------- phase 1: projections ----------------
            with (
                tc.tile_pool(name="psq", bufs=1, space="PSUM") as psq,
                tc.tile_pool(name="psv", bufs=1, space="PSUM") as psv,
                tc.tile_pool(name="xs", bufs=2) as xs,
                tc.tile_pool(name="xtmp", bufs=2) as xtmp,
            ):
                for qb in range(NQB):
                    qsl = slice(qb * QB, (qb + 1) * QB)
                    xh = []
                    xdr = []
                    for di in range(DCH):
                        dsl = slice(di * P, (di + 1) * P)
                        xht = xs.tile([P, QB], fp16, tag=f"xh{di}",
                                      name=f"xh{di}")
                        nc.sync.dma_start(xht[:], xh16d[dsl, qsl])
                        xdt = xs.tile([P, 2, QB], e4m3, tag=f"xd{di}",
                                      name=f"xd{di}")
                        nc.sync.dma_start(xdt[:], xdrd[dsl, :, qsl])
                        xh.append(xht)
                        xdr.append(xdt)
                    if qb == 0:
                        nc.sync.dma_start(
                            woh_sb[:], woh.rearrange("(dc p) c -> p dc c",
                                                     p=P))
                    for nm, whs, wds, hi_t, dr_t in (
                            ("q", wqh_sb, wqdr_sb, QH16, QDR),
                            ("k", wkh_sb, wkdr_sb, KH16, KDR)):
                        dscale = DRAINQ if nm == "q" else DRAINK
                        for cc in range(NCC):
                            csl = slice(cc * P, (cc + 1) * P)
                            acc = psq.tile([P, QB], f32, tag=f"a{nm}{cc}",
                                           name=f"a{nm}{cc}")
                            for di in range(DCH):
                                nc.tensor.matmul(
                                    acc[:], whs[:, di, csl], xh[di][:],
                                    start=(di == 0), stop=False)
                                nc.tensor.matmul(
                                    acc[:], wds[:, di, :, csl], xdr[di][:],
                                    start=False, stop=(di == DCH - 1),
                                    perf_mode=DR)
                            nc.scalar.activation(hi_t[:, cc, qsl], acc[:],
                                                 COPYF, scale=dscale)
                            if nm == "q":
                                # QL8 = e4m3(128 Ql): STT gives 32Ql in f32,
                                # then ACT rescales x4 into e4m3
                                t32 = xtmp.tile([P, QB], f32, tag="t32",
                                                name="t32")
                                nc.vector.scalar_tensor_tensor(
                                    t32[:], acc[:], dscale,
                                    hi_t[:, cc, qsl], MULT, SUB)
                                nc.scalar.activation(
                                    dr_t[:, cc, 1, qsl], t32[:], COPYF,
                                    scale=4.0)
                                # QH8 = e4m3(32 Qh): plain cast of QH16
                                nc.vector.tensor_copy(
                                    dr_t[:, cc, 0, qsl], hi_t[:, cc, qsl])
                            else:
                                # KL8 = e4m3(4 Kl)
                                nc.vector.scalar_tensor_tensor(
                                    dr_t[:, cc, 0, qsl], acc[:], dscale,
                                    hi_t[:, cc, qsl], MULT, SUB)
                                # KH8 = e4m3(Kh) = KH16 x 1/4
                                nc.scalar.activation(
                                    dr_t[:, cc, 1, qsl], hi_t[:, cc, qsl],
                                    COPYF, scale=0.25)
                    for r in range(KPB):
                        vacc = psv.tile([P, CPC], f32, tag=f"pv{r}",
                                        name=f"pv{r}")
                        for di in range(DCH):
                            nc.tensor.matmul(
                                vacc[:], xh[di][:, r * P:(r + 1) * P],
                                wvh_sb[:, di, :],
                                start=(di == 0), stop=(di == DCH - 1))
                        ki = qb * KPB + r
                        dst = Vsb[:, ki].rearrange(
                            "p (g w) -> p g w", w=VW)[:, :, 0:64]
                        src = vacc[:].rearrange("p (g w) -> p g w", w=64)
                        nc.scalar.activation(dst, src, COPYF)

            # ---------------- phase 2 + 3, software-pipelined ----------------
            with (
                tc.tile_pool(name="pssc", bufs=4, space="PSUM") as pssc,
                tc.tile_pool(name="psot", bufs=2, space="PSUM") as psot,
                tc.tile_pool(name="pso", bufs=1, space="PSUM") as pso,
                tc.tile_pool(name="stgp", bufs=34) as stgp,
                tc.tile_pool(name="expp", bufs=8) as expp,
            ):
                def phase3_block(j):
                    """out-proj for q block j (consumes that block's gather)."""
                    qsl3 = slice(j * QB, (j + 1) * QB)
                    accs = [pso.tile([P, QB], f32, tag=f"po{occ}",
                                     name=f"po{occ}")
                            for occ in range(NCC)]
                    last = (j == NQB - 1)
                    # cc-major order so the last block's late gathers (hp=1)
                    # are needed as late as possible
                    order = (sorted(range(DCH), key=lambda m: (m % NCC, m // NCC))
                             if last else list(range(DCH)))
                    for i, mch in enumerate(order):
                        g_, cc_ = mch // NCC, mch % NCC
                        mt = ms.tile([P, QB], fp16, tag="mt", name="mt")
                        if last:
                            nc.sync.dma_start(mt[0:64, :],
                                              ag_out[j, cc_, 0][g_])
                            nc.sync.dma_start(mt[64:128, :],
                                              ag_out[j, cc_, 1][g_])
                        else:
                            nc.sync.dma_start(mt[:], ag_out[j, cc_][g_])
                        for occ in range(NCC):
                            nc.tensor.matmul(
                                accs[occ][:],
                                woh_sb[:, mch, occ * P:(occ + 1) * P],
                                mt[:], start=(i == 0), stop=(i == DCH - 1))
                    for occ in range(NCC):
                        oo = op.tile([P, QB], f32, tag="oo", name="oo")
                        nc.any.tensor_copy(oo[:], accs[occ][:])
                        nc.sync.dma_start(outT[occ, :, qsl3], oo[:])

                def _gather(inp, outp):
                    nc.gpsimd.collective_compute(
                        "AllGather", mybir.AluOpType.bypass,
                        replica_groups=[[0, 1, 2, 3], [4, 5, 6, 7]],
                        ins=[inp], outs=[outp],
                    )

                def emit_A_chunk(st, kc):
                    qb, hp, h2, sts, rm, _ = st
                    hsl = slice(h2 * 64, (h2 + 1) * 64)
                    diag = kc - qb * KPB
                    off = max(0, diag) * P
                    psc = pssc.tile([P, QB], f32, tag="psc", name="psc")
                    ksl = slice(kc * P, (kc + 1) * P)
                    mvsl = slice(qb * QB + off, (qb + 1) * QB)
                    nc.tensor.matmul(
                        psc[:, off:], KH16[hsl, hp, ksl],
                        QH16[hsl, hp, mvsl], start=True, stop=False)
                    nc.tensor.matmul(
                        psc[:, off:], KDR[hsl, hp, :, ksl],
                        QDR[hsl, hp, :, mvsl], start=False,
                        stop=(diag < 0), perf_mode=DR)
                    if diag >= 0:
                        nc.tensor.matmul(
                            psc[:, off:off + P], msk_st[:],
                            msk_mv[:], start=False, stop=True)
                    stg = stgp.tile([P, QB], f32, tag="stg", name="stg")
                    nc.scalar.activation(stg[:, off:], psc[:, off:], COPYF)
                    nc.vector.tensor_tensor(rm[:, off:], rm[:, off:],
                                            stg[:, off:], MAXOP)
                    sts.append((stg, off))

                def emit_B_chunk(st, kc):
                    qb, hp, h2, sts, _, ctx = st
                    otp, mrep = ctx
                    vg = slice((hp * 2 + h2) * VW, (hp * 2 + h2 + 1) * VW)
                    nkc = len(sts)
                    stg, off = sts[kc]
                    nc.vector.tensor_tensor(stg[:, off:], stg[:, off:],
                                            mrep[:, off:], SUB)
                    ex = expp.tile([P, QB], fp16, tag="ex", name="ex")
                    nc.scalar.activation(ex[:, off:], stg[:, off:], EXP,
                                         bias=mbias[:], scale=1.0 / SSCORE)
                    nc.tensor.matmul(otp[:, off:], Vsb[:, kc, vg],
                                     ex[:, off:],
                                     start=(kc == 0), stop=(kc == nkc - 1))

                def finish_B(st):
                    qb, hp, h2, sts, _, ctx = st
                    otp, mrep = ctx
                    qsl = slice(qb * QB, (qb + 1) * QB)
                    hsl = slice(h2 * 64, (h2 + 1) * 64)
                    lsb = stat.tile([1, QB], f32, tag="lsb", name="lsb")
                    nc.vector.tensor_copy(lsb[:], otp[64:65, :])
                    rec = stat.tile([1, QB], f32, tag="rec", name="rec")
                    nc.vector.reciprocal_approx_fast(rec[:], lsb[:])
                    recb = stat.tile([64, QB], f32, tag="recb", name="recb")
                    nc.gpsimd.partition_broadcast(recb[:], rec[:], 64)
                    nc.vector.tensor_tensor(OT[hsl, hp, qsl], otp[0:64, :],
                                            recb[:], MULT)
                    # fire gathers / interleaved out-proj on block boundaries
                    if qb == NQB - 1:
                        nc.sync.dma_start(ag_in[qb, hp, h2][:],
                                          OT[hsl, hp, qsl])
                        _gather(ag_in[qb, hp, h2][:], ag_out[qb, hp, h2][:])
                        if hp == 0 and h2 == 1 and qb >= 1:
                            phase3_block(qb - 1)
                    elif h2 == 1:
                        nc.sync.dma_start(ag_in[qb, hp][:], OT[:, hp, qsl])
                        _gather(ag_in[qb, hp][:], ag_out[qb, hp][:])
                        if hp == NCC - 1 and qb >= 1:
                            phase3_block(qb - 1)

                blocks = [(qb, hp, h2) for qb in range(NQB)
                          for hp in range(NCC) for h2 in range(2)]
                prev = None
                for qb, hp, h2 in blocks:
                    rm = stat.tile([P, QB], bf16, tag="rm", name="rm")
                    nc.gpsimd.memset(rm[:], -3e38)
                    cur = (qb, hp, h2, [], rm, None)
                    nA = qb * KPB + KPB
                    nB = len(prev[3]) if prev else 0
                    for i in range(max(nA, nB)):
                        if i < nA:
                            emit_A_chunk(cur, i)
                        if i < nB:
                            emit_B_chunk(prev, i)
                    if prev is not None:
                        finish_B(prev)
                    mrep = stat.tile([P, QB], bf16, tag="mrep", name="mrep")
                    nc.gpsimd.partition_all_reduce(
                        mrep[:], rm[:], P, bass_isa.ReduceOp.max)
                    otp = psot.tile([VW, QB], f32, tag="otp", name="otp")
                    prev = (qb, hp, h2, cur[3], rm, (otp, mrep))
                for i in range(len(prev[3])):
                    emit_B_chunk(prev, i)
                finish_B(prev)
                phase3_block(NQB - 1)

    nc.compile()
    return nc


_NC_CACHE = {}


def get_nc(**cfg):
    key = tuple(sorted(cfg.items()))
    if key not in _NC_CACHE:
        _NC_CACHE[key] = build_nc(**cfg)
    return _NC_CACHE[key]


def _col_index(g):
    p = np.arange(CPC)
    return (p % HD) * HEADS + (HPC * g + p // HD)


def _ow_row_index():
    r = np.arange(D)
    m, p128 = r // P, r % P
    g_, cc = m // NCC, m % NCC
    p256 = cc * P + p128
    lh, hd = p256 // HD, p256 % HD
    return hd * HEADS + (HPC * g_ + lh)


def _split16(w, shi):
    """w -> (fp16(shi*w), e4m3(shi*(w-hi/shi)), e4m3(shi/16*whi))"""
    hi = (shi * w).astype(np.float16)
    lo = shi * w - hi.astype(np.float32)
    l8 = lo.astype(ml_dtypes.float8_e4m3)
    h8 = (hi.astype(np.float32) / 16.0).astype(ml_dtypes.float8_e4m3)
    return hi, l8, h8


def make_in_maps(x, qw, kw, vw, ow, s=S):
    scale = 1.0 / np.sqrt(np.float32(D))
    qws = (qw * scale).astype(np.float32)
    ow_perm = np.ascontiguousarray(ow[_ow_row_index()])

    # x hi/lo splits, shared per batch
    xsplits = []
    for b in range(B):
        xT = np.ascontiguousarray(x[b, :s].T).astype(np.float32)
        xh = (SX * xT).astype(np.float16)          # fp16(16 x)
        xl = SX * xT - xh.astype(np.float32)       # 16 xl
        xdr = np.empty((D, 2, s), dtype=ml_dtypes.float8_e4m3)
        xdr[:, 0, :] = xh.astype(ml_dtypes.float8_e4m3)   # e4m3(16 xh)
        xdr[:, 1, :] = (SXL / SX * xl).astype(ml_dtypes.float8_e4m3)
        xsplits.append((xh, xdr))

    mskst = (-1e10 * np.eye(P, dtype=np.float32)).astype(ml_dtypes.bfloat16)
    mskmv = np.tril(np.ones((P, P), dtype=np.float32), -1).astype(
        ml_dtypes.bfloat16)

    in_maps = []
    for c in range(N_CORES):
        b, g = c // GROUPS, c % GROUPS
        cols = _col_index(g)
        wq = np.ascontiguousarray(qws[:, cols])
        wk = np.ascontiguousarray(kw[:, cols]).astype(np.float32)
        qh, ql8, qh8 = _split16(wq, SWQ)
        kh, kl8, kh8 = _split16(wk, SWK)
        wqdr = np.empty((D, 2, CPC), dtype=ml_dtypes.float8_e4m3)
        wqdr[:, 0, :] = ql8
        wqdr[:, 1, :] = qh8
        wkdr = np.empty((D, 2, CPC), dtype=ml_dtypes.float8_e4m3)
        wkdr[:, 0, :] = kl8
        wkdr[:, 1, :] = kh8
        xh, xdr = xsplits[b]
        in_maps.append({
            "xh16": xh,
            "xdr": xdr,
            "wqh": qh,
            "wqdr": wqdr,
            "wkh": kh,
            "wkdr": wkdr,
            "wvh": (np.ascontiguousarray(vw[:, cols]) / SX).astype(
                np.float16),
            "woh": np.ascontiguousarray(
                ow_perm[:, g * CPC:(g + 1) * CPC]).astype(np.float16),
            "mskst": mskst,
            "mskmv": mskmv,
        })
    return in_maps


def assemble_output(results, s=S):
    out = np.empty((B, s, D), dtype=np.float32)
    for c in range(N_CORES):
        b, g = c // GROUPS, c % GROUPS
        oT = results[c]["outT"]  # [NCC, P, s]
        for occ in range(NCC):
            out[b, :, g * CPC + occ * P:(g * CPC + (occ + 1) * P)] = oT[occ].T
    return out


def run_on_hw(x, qw, kw, vw, ow, trace=False, **cfg_over):
    cfg = dict(DEFAULT_CFG)
    cfg.update(cfg_over)
    s = cfg["s"]
    nc = get_nc(**cfg)
    in_maps = make_in_maps(x, qw, kw, vw, ow, s=s)
    res = run_bass_kernel_spmd(nc, in_maps, core_ids=list(range(N_CORES)),
                               trace=trace)
    return assemble_output(res.results, s=s), res


def kernel(x, qw, kw, vw, ow):
    out, _ = run_on_hw(np.asarray(x, dtype=np.float32),
                       np.asarray(qw, dtype=np.float32),
                       np.asarray(kw, dtype=np.float32),
                       np.asarray(vw, dtype=np.float32),
                       np.asarray(ow, dtype=np.float32))
    return out


# revision 20
# speedup vs baseline: 1.2420x; 1.0065x over previous
"""Multi-head attention (dense_transformer) on 8 TRN2 NeuronCores.

Sharding: 2-way data parallel over batch x 4-way tensor parallel over heads.
Core c handles batch b=c//4 and heads {4g..4g+3} where g=c%4 (4 heads, 256
channels per core; channels of head h are qw columns {hd*16+h}).

Architecture v3 ("fp16 + fp8-DoubleRow pseudo-fp32"):
  The pseudo-fp32 matmuls (Q/K projections and K^T Q scores) need ~15-bit
  operand mantissas because score std is ~256 and softmax is near-argmax.
  v2 used 3-term bf16 splits (3 passes).  v3 uses:
    main term:  fp16 x fp16 (11-bit mantissas), 1.0 PE cycles/column
    both cross terms: one fp8-e4m3 matmul in DoubleRow perf mode (two
      stationary/moving stream pairs summed into one PSUM output) at 0.5
      cycles/column.
  => 1.5 pass-equivalents instead of 3.  Per-stream power-of-2 scales keep
  every operand inside e4m3/fp16 range; all passes of one accumulation
  produce the same product scale, which is folded into the EXP scale.

  Score-side operand scales (score PSUM = 128*score; every e4m3 operand
  stays under 224 so the e4m3/e4m3fn variant ambiguity is moot):
    KH16=fp16(4K), KL8=e4m3(4(K-Kh)), KH8=e4m3(Kh)
    QH16=fp16(32Q), QL8=e4m3(128(Q-Qh)), QH8=e4m3(32Qh)
    DR streams: (KL8,QH8) + (KH8,QL8) -> 128*(Kl Qh + Kh Ql)
  Projection pass scales: QPSUM=32768*Q (wq folded 1/32), KPSUM=1024*K;
  both drain with the same ActCopy scale 1/512.

  x and weight hi/lo splits are precomputed on the HOST (no on-device
  split work; x ships as fp16 + e4m3 pair, same bytes as one f32 copy).

  Causal mask is applied on the PE: an extra rank-structured matmul term
  (stationary -1e10*I bf16, moving strictly-lower-triangular ones) adds
  -1e10 to masked elements of the diagonal chunks inside PSUM, before any
  drain -- no DVE masking.

  V projection: single fp16 pass (xh * fp16(wv/16)); V, exp weights, OT,
  and the out-projection all run in fp16 (11-bit) instead of bf16, which
  *reduces* error vs v2 while the out-projection drops to a single pass.

  Softmax sum reciprocal: vector-engine reciprocal_approx_fast (1 DVE op)
  instead of Ln+Exp on ACT (kills the activation-table thrash).

  Engine balance per score chunk: PSUM drain (ACT Copy), running max
  (DVE), subtract (DVE), EXP (ACT).
"""
import sys

sys.path.insert(0, "/opt/trn_rl_repo")

import numpy as np
import ml_dtypes

import concourse.bass as bass
import concourse.mybir as mybir
import concourse.tile as tile
from concourse import bacc
from concourse import bass_isa
from concourse.bass_utils import run_bass_kernel_spmd

# ---- problem constants (hardcoded per harness contract) ----
B, S, D, HEADS = 2, 2048, 1024, 16
N_CORES = 8
GROUPS = 4                 # head-groups == cores per batch
HPC = HEADS // GROUPS      # heads per core (4)
HD = D // HEADS            # 64
CPC = HPC * HD             # channels per core (256)
P = 128
NCC = CPC // P             # col chunks per core (2)
DCH = D // P               # contraction chunks (8)
QB = 512                   # q block width (1 PSUM bank of f32)

f32 = mybir.dt.float32
bf16 = mybir.dt.bfloat16
fp16 = mybir.dt.float16
e4m3 = mybir.dt.float8e4

AX = mybir.AxisListType
EXP = mybir.ActivationFunctionType.Exp
MAXOP = mybir.AluOpType.max
SUB = mybir.AluOpType.subtract
MULT = mybir.AluOpType.mult
COPYF = mybir.ActivationFunctionType.Copy
DR = mybir.MatmulPerfMode.DoubleRow

DEFAULT_CFG = dict(s=S)

# host-side split scales
SWQ = 2048.0   # wq-tilde (=qw/32) fp16-hi scale
SWK = 64.0     # kw fp16-hi scale
SX = 16.0      # x fp16-hi scale
SXL = 256.0    # x e4m3-lo scale
DRAINQ = 1.0 / 1024.0  # QPSUM(32768Q) -> fp16(32Q)
DRAINK = 1.0 / 256.0   # KPSUM(1024K) -> fp16(4K)
SSCORE = 128.0         # score PSUM scale


def build_nc(s=S, dbg=False):
    assert s % QB == 0
    NQB = s // QB            # 512-wide q blocks
    NKC = s // P             # 128-wide k chunks
    KPB = QB // P            # k chunks per q block on the diagonal (4)
    VW = 65                  # V channels per (hp,h2) incl the ones column
    NH2 = NCC * 2            # head slots per core (4)

    nc = bacc.Bacc("TRN2", target_bir_lowering=False, debug=False,
                   num_devices=N_CORES)
    xh16d = nc.dram_tensor("xh16", [D, s], fp16, kind="ExternalInput").ap()
    xdrd = nc.dram_tensor("xdr", [D, 2, s], e4m3, kind="ExternalInput").ap()
    wqh = nc.dram_tensor("wqh", [D, CPC], fp16, kind="ExternalInput").ap()
    wqdr = nc.dram_tensor("wqdr", [D, 2, CPC], e4m3, kind="ExternalInput").ap()
    wkh = nc.dram_tensor("wkh", [D, CPC], fp16, kind="ExternalInput").ap()
    wkdr = nc.dram_tensor("wkdr", [D, 2, CPC], e4m3, kind="ExternalInput").ap()
    wvh = nc.dram_tensor("wvh", [D, CPC], fp16, kind="ExternalInput").ap()
    woh = nc.dram_tensor("woh", [D, CPC], fp16, kind="ExternalInput").ap()
    mskst = nc.dram_tensor("mskst", [P, P], bf16, kind="ExternalInput").ap()
    mskmv = nc.dram_tensor("mskmv", [P, P], bf16, kind="ExternalInput").ap()
    outT = nc.dram_tensor("outT", [NCC, P, s], f32, kind="ExternalOutput").ap()

    with tile.TileContext(nc) as tc:
        with (
            tc.tile_pool(name="cpool", bufs=1) as cpool,
            tc.tile_pool(name="wpool", bufs=1) as wpool,
            tc.tile_pool(name="big", bufs=1) as big,
            tc.tile_pool(name="stat", bufs=3) as stat,
            tc.tile_pool(name="ms", bufs=6) as ms,
            tc.tile_pool(name="op", bufs=2) as op,
            tc.tile_pool(name="dram", bufs=1, space="DRAM") as dpool,
        ):
            # per-(qb,hp) gather buffers; last qb additionally split by h2
            ag_in = {}
            ag_out = {}
            for qb in range(NQB):
                for hp in range(NCC):
                    if qb == NQB - 1:
                        for h2 in range(2):
                            ag_in[qb, hp, h2] = dpool.tile(
                                [64, QB], fp16, tag=f"agi{qb}_{hp}_{h2}",
                                name=f"agi{qb}_{hp}_{h2}")
                            ag_out[qb, hp, h2] = dpool.tile(
                                [GROUPS, 64, QB], fp16,
                                tag=f"ago{qb}_{hp}_{h2}",
                                name=f"ago{qb}_{hp}_{h2}")
                    else:
                        ag_in[qb, hp] = dpool.tile(
                            [P, QB], fp16, tag=f"agi{qb}_{hp}",
                            name=f"agi{qb}_{hp}")
                        ag_out[qb, hp] = dpool.tile(
                            [GROUPS, P, QB], fp16, tag=f"ago{qb}_{hp}",
                            name=f"ago{qb}_{hp}")

            # weights + constants in SBUF
            msk_st = cpool.tile([P, P], bf16, tag="mskst")
            msk_mv = cpool.tile([P, P], bf16, tag="mskmv")
            wqh_sb = wpool.tile([P, DCH, CPC], fp16, tag="wqh")
            wqdr_sb = wpool.tile([P, DCH, 2, CPC], e4m3, tag="wqdr")
            wkh_sb = wpool.tile([P, DCH, CPC], fp16, tag="wkh")
            wkdr_sb = wpool.tile([P, DCH, 2, CPC], e4m3, tag="wkdr")
            wvh_sb = wpool.tile([P, DCH, CPC], fp16, tag="wvh")
            woh_sb = wpool.tile([P, DCH, CPC], fp16, tag="woh")
            # one DMA per weight tensor ([D,...] viewed as [P, DCH, ...]);
            # wo deferred until after qb0's x tiles (not needed in phase 1)
            nc.sync.dma_start(
                wqh_sb[:], wqh.rearrange("(dc p) c -> p dc c", p=P))
            nc.sync.dma_start(
                wqdr_sb[:], wqdr.rearrange("(dc p) t c -> p dc t c", p=P))
            nc.sync.dma_start(
                wkh_sb[:], wkh.rearrange("(dc p) c -> p dc c", p=P))
            nc.sync.dma_start(
                wkdr_sb[:], wkdr.rearrange("(dc p) t c -> p dc t c", p=P))
            nc.sync.dma_start(msk_st[:], mskst)
            nc.sync.dma_start(msk_mv[:], mskmv)
            nc.sync.dma_start(
                wvh_sb[:], wvh.rearrange("(dc p) c -> p dc c", p=P))

            QH16 = big.tile([P, NCC, s], fp16, tag="QH16")
            QDR = big.tile([P, NCC, 2, s], e4m3, tag="QDR")
            KH16 = big.tile([P, NCC, s], fp16, tag="KH16")
            KDR = big.tile([P, NCC, 2, s], e4m3, tag="KDR")
            # Vhat[k, :]: 4 groups of 65 cols: 64 V channels + a ones col
            Vsb = big.tile([P, NKC, NH2 * VW], fp16, tag="Vsb")
            OT = big.tile([P, NCC, s], fp16, tag="OT")

            for g in range(NH2):
                nc.gpsimd.memset(Vsb[:, :, g * VW + 64], 1.0)
            mbias = cpool.tile([P, 1], f32, tag="mbias")
            nc.gpsimd.memset(mbias[:], -6.0)

            # ---------------- phase 1: projections ----------------
            with (
                tc.tile_pool(name="psq", bufs=1, space="PSUM") as psq,
                tc.tile_pool(name="psv", bufs=1, space="PSUM") as psv,
                tc.tile_pool(name="xs", bufs=2) as xs,
                tc.tile_pool(name="xtmp", bufs=2) as xtmp,
            ):
                for qb in range(NQB):
                    qsl = slice(qb * QB, (qb + 1) * QB)
                    xh = []
                    xdr = []
                    for di in range(DCH):
                        dsl = slice(di * P, (di + 1) * P)
                        xht = xs.tile([P, QB], fp16, tag=f"xh{di}",
                                      name=f"xh{di}")
                        nc.sync.dma_start(xht[:], xh16d[dsl, qsl])
                        xdt = xs.tile([P, 2, QB], e4m3, tag=f"xd{di}",
                                      name=f"xd{di}")
                        nc.sync.dma_start(xdt[:], xdrd[dsl, :, qsl])
                        xh.append(xht)
                        xdr.append(xdt)
                    if qb == 0:
                        nc.sync.dma_start(
                            woh_sb[:], woh.rearrange("(dc p) c -> p dc c",
                                                     p=P))
                    for nm, whs, wds, hi_t, dr_t in (
                            ("q", wqh_sb, wqdr_sb, QH16, QDR),
                            ("k", wkh_sb, wkdr_sb, KH16, KDR)):
                        dscale = DRAINQ if nm == "q" else DRAINK
                        for cc in range(NCC):
                            csl = slice(cc * P, (cc + 1) * P)
                            acc = psq.tile([P, QB], f32, tag=f"a{nm}{cc}",
                                           name=f"a{nm}{cc}")
                            for di in range(DCH):
                                nc.tensor.matmul(
                                    acc[:], whs[:, di, csl], xh[di][:],
                                    start=(di == 0), stop=False)
                                nc.tensor.matmul(
                                    acc[:], wds[:, di, :, csl], xdr[di][:],
                                    start=False, stop=(di == DCH - 1),
                                    perf_mode=DR)
                            nc.scalar.activation(hi_t[:, cc, qsl], acc[:],
                                                 COPYF, scale=dscale)
                            if nm == "q":
                                # QL8 = e4m3(128 Ql): STT gives 32Ql in f32,
                                # then ACT rescales x4 into e4m3
                                t32 = xtmp.tile([P, QB], f32, tag="t32",
                                                name="t32")
                                nc.vector.scalar_tensor_tensor(
                                    t32[:], acc[:], dscale,
                                    hi_t[:, cc, qsl], MULT, SUB)
                                nc.scalar.activation(
                                    dr_t[:, cc, 1, qsl], t32[:], COPYF,
                                    scale=4.0)
                                # QH8 = e4m3(32 Qh): plain cast of QH16
                                nc.vector.tensor_copy(
                                    dr_t[:, cc, 0, qsl], hi_t[:, cc, qsl])
                            else:
                                # KL8 = e4m3(4 Kl)
                                nc.vector.scalar_tensor_tensor(
                                    dr_t[:, cc, 0, qsl], acc[:], dscale,
                                    hi_t[:, cc, qsl], MULT, SUB)
                                # KH8 = e4m3(Kh) = KH16 x 1/4
                                nc.scalar.activation(
                                    dr_t[:, cc, 1, qsl], hi_t[:, cc, qsl],
                                    COPYF, scale=0.25)
                    for r in range(KPB):
                        vacc = psv.tile([P, CPC], f32, tag=f"pv{r}",
                                        name=f"pv{r}")
                        for di in range(DCH):
                            nc.tensor.matmul(
                                vacc[:], xh[di][:, r * P:(r + 1) * P],
                                wvh_sb[:, di, :],
                                start=(di == 0), stop=(di == DCH - 1))
                        ki = qb * KPB + r
                        dst = Vsb[:, ki].rearrange(
                            "p (g w) -> p g w", w=VW)[:, :, 0:64]
                        src = vacc[:].rearrange("p (g w) -> p g w", w=64)
                        nc.scalar.activation(dst, src, COPYF)

            # ---------------- phase 2 + 3, software-pipelined ----------------
            with (
                tc.tile_pool(name="pssc", bufs=4, space="PSUM") as pssc,
                tc.tile_pool(name="psot", bufs=2, space="PSUM") as psot,
                tc.tile_pool(name="pso", bufs=1, space="PSUM") as pso,
                tc.tile_pool(name="stgp", bufs=34) as stgp,
                tc.tile_pool(name="expp", bufs=8) as expp,
            ):
                def phase3_block(j):
                    """out-proj for q block j (consumes that block's gather)."""
                    qsl3 = slice(j * QB, (j + 1) * QB)
                    accs = [pso.tile([P, QB], f32, tag=f"po{occ}",
                                     name=f"po{occ}")
                            for occ in range(NCC)]
                    last = (j == NQB - 1)
                    # cc-major order so the last block's late gathers (hp=1)
                    # are needed as late as possible
                    order = (sorted(range(DCH), key=lambda m: (m % NCC, m // NCC))
                             if last else list(range(DCH)))
                    for i, mch in enumerate(order):
                        g_, cc_ = mch // NCC, mch % NCC
                        mt = ms.tile([P, QB], fp16, tag="mt", name="mt")
                        if last:
                            nc.sync.dma_start(mt[0:64, :],
                                              ag_out[j, cc_, 0][g_])
                            nc.sync.dma_start(mt[64:128, :],
                                              ag_out[j, cc_, 1][g_])
                        else:
                            nc.sync.dma_start(mt[:], ag_out[j, cc_][g_])
                        for occ in range(NCC):
                            nc.tensor.matmul(
                                accs[occ][:],
                                woh_sb[:, mch, occ * P:(occ + 1) * P],
                                mt[:], start=(i == 0), stop=(i == DCH - 1))
                    for occ in range(NCC):
                        oo = op.tile([P, QB], f32, tag="oo", name="oo")
                        nc.any.tensor_copy(oo[:], accs[occ][:])
                        nc.sync.dma_start(outT[occ, :, qsl3], oo[:])

                def _gather(inp, outp):
                    nc.gpsimd.collective_compute(
                        "AllGather", mybir.AluOpType.bypass,
                        replica_groups=[[0, 1, 2, 3], [4, 5, 6, 7]],
                        ins=[inp], outs=[outp],
                    )

                def emit_A_chunk(st, kc):
                    qb, hp, h2, sts, rm, _ = st
                    hsl = slice(h2 * 64, (h2 + 1) * 64)
                    diag = kc - qb * KPB
                    off = max(0, diag) * P
                    psc = pssc.tile([P, QB], f32, tag="psc", name="psc")
                    ksl = slice(kc * P, (kc + 1) * P)
                    mvsl = slice(qb * QB + off, (qb + 1) * QB)
                    nc.tensor.matmul(
                        psc[:, off:], KH16[hsl, hp, ksl],
                        QH16[hsl, hp, mvsl], start=True, stop=False)
                    nc.tensor.matmul(
                        psc[:, off:], KDR[hsl, hp, :, ksl],
                        QDR[hsl, hp, :, mvsl], start=False,
                        stop=(diag < 0), perf_mode=DR)
                    if diag >= 0:
                        nc.tensor.matmul(
                            psc[:, off:off + P], msk_st[:],
                            msk_mv[:], start=False, stop=True)
                    stg = stgp.tile([P, QB], f32, tag="stg", name="stg")
                    nc.scalar.activation(stg[:, off:], psc[:, off:], COPYF)
                    if kc == 0:
                        nc.vector.tensor_copy(rm[:], stg[:])
                    else:
                        nc.vector.tensor_tensor(rm[:, off:], rm[:, off:],
                                                stg[:, off:], MAXOP)
                    sts.append((stg, off))

                def emit_B_chunk(st, kc):
                    qb, hp, h2, sts, _, ctx = st
                    otp, mrep = ctx
                    vg = slice((hp * 2 + h2) * VW, (hp * 2 + h2 + 1) * VW)
                    nkc = len(sts)
                    stg, off = sts[kc]
                    nc.vector.tensor_tensor(stg[:, off:], stg[:, off:],
                                            mrep[:, off:], SUB)
                    ex = expp.tile([P, QB], fp16, tag="ex", name="ex")
                    nc.scalar.activation(ex[:, off:], stg[:, off:], EXP,
                                         bias=mbias[:], scale=1.0 / SSCORE)
                    nc.tensor.matmul(otp[:, off:], Vsb[:, kc, vg],
                                     ex[:, off:],
                                     start=(kc == 0), stop=(kc == nkc - 1))

                def finish_B(st):
                    qb, hp, h2, sts, _, ctx = st
                    otp, mrep = ctx
                    qsl = slice(qb * QB, (qb + 1) * QB)
                    hsl = slice(h2 * 64, (h2 + 1) * 64)
                    lsb = stat.tile([1, QB], f32, tag="lsb", name="lsb")
                    nc.vector.tensor_copy(lsb[:], otp[64:65, :])
                    rec = stat.tile([1, QB], f32, tag="rec", name="rec")
                    nc.vector.reciprocal_approx_fast(rec[:], lsb[:])
                    recb = stat.tile([64, QB], f32, tag="recb", name="recb")
                    nc.gpsimd.partition_broadcast(recb[:], rec[:], 64)
                    nc.vector.tensor_tensor(OT[hsl, hp, qsl], otp[0:64, :],
                                            recb[:], MULT)

                def fire_gather(st):
                    """gather trigger + out-proj injection, deferred so the
                    collective trigger's gpsimd-queue stall lands after the
                    next block's partition_all_reduce."""
                    qb, hp, h2, _, _, _ = st
                    qsl = slice(qb * QB, (qb + 1) * QB)
                    hsl = slice(h2 * 64, (h2 + 1) * 64)
                    if qb == NQB - 1:
                        nc.sync.dma_start(ag_in[qb, hp, h2][:],
                                          OT[hsl, hp, qsl])
                        _gather(ag_in[qb, hp, h2][:], ag_out[qb, hp, h2][:])
                        if hp == 0 and h2 == 1 and qb >= 1:
                            phase3_block(qb - 1)
                    elif h2 == 1:
                        nc.sync.dma_start(ag_in[qb, hp][:], OT[:, hp, qsl])
                        _gather(ag_in[qb, hp][:], ag_out[qb, hp][:])
                        if hp == NCC - 1 and qb >= 1:
                            phase3_block(qb - 1)

                blocks = [(qb, hp, h2) for qb in range(NQB)
                          for hp in range(NCC) for h2 in range(2)]
                prev = None
                for qb, hp, h2 in blocks:
                    rm = stat.tile([P, QB], bf16, tag="rm", name="rm")
                    cur = (qb, hp, h2, [], rm, None)
                    nA = qb * KPB + KPB
                    nB = len(prev[3]) if prev else 0
                    for i in range(max(nA, nB)):
                        if i < nA:
                            emit_A_chunk(cur, i)
                        if i < nB:
                            emit_B_chunk(prev, i)
                    if prev is not None:
                        finish_B(prev)
                    mrep = stat.tile([P, QB], bf16, tag="mrep", name="mrep")
                    nc.gpsimd.partition_all_reduce(
                        mrep[:], rm[:], P, bass_isa.ReduceOp.max)
                    if prev is not None:
                        fire_gather(prev)
                    otp = psot.tile([VW, QB], f32, tag="otp", name="otp")
                    prev = (qb, hp, h2, cur[3], rm, (otp, mrep))
                for i in range(len(prev[3])):
                    emit_B_chunk(prev, i)
                finish_B(prev)
                fire_gather(prev)
                phase3_block(NQB - 1)

    nc.compile()
    return nc


_NC_CACHE = {}


def get_nc(**cfg):
    key = tuple(sorted(cfg.items()))
    if key not in _NC_CACHE:
        _NC_CACHE[key] = build_nc(**cfg)
    return _NC_CACHE[key]


def _col_index(g):
    p = np.arange(CPC)
    return (p % HD) * HEADS + (HPC * g + p // HD)


def _ow_row_index():
    r = np.arange(D)
    m, p128 = r // P, r % P
    g_, cc = m // NCC, m % NCC
    p256 = cc * P + p128
    lh, hd = p256 // HD, p256 % HD
    return hd * HEADS + (HPC * g_ + lh)


def _split16(w, shi):
    """w -> (fp16(shi*w), e4m3(shi*(w-hi/shi)), e4m3(shi/16*whi))"""
    hi = (shi * w).astype(np.float16)
    lo = shi * w - hi.astype(np.float32)
    l8 = lo.astype(ml_dtypes.float8_e4m3)
    h8 = (hi.astype(np.float32) / 16.0).astype(ml_dtypes.float8_e4m3)
    return hi, l8, h8


def make_in_maps(x, qw, kw, vw, ow, s=S):
    scale = 1.0 / np.sqrt(np.float32(D))
    qws = (qw * scale).astype(np.float32)
    ow_perm = np.ascontiguousarray(ow[_ow_row_index()])

    # x hi/lo splits, shared per batch
    xsplits = []
    for b in range(B):
        xT = np.ascontiguousarray(x[b, :s].T).astype(np.float32)
        xh = (SX * xT).astype(np.float16)          # fp16(16 x)
        xl = SX * xT - xh.astype(np.float32)       # 16 xl
        xdr = np.empty((D, 2, s), dtype=ml_dtypes.float8_e4m3)
        xdr[:, 0, :] = xh.astype(ml_dtypes.float8_e4m3)   # e4m3(16 xh)
        xdr[:, 1, :] = (SXL / SX * xl).astype(ml_dtypes.float8_e4m3)
        xsplits.append((xh, xdr))

    mskst = (-1e10 * np.eye(P, dtype=np.float32)).astype(ml_dtypes.bfloat16)
    mskmv = np.tril(np.ones((P, P), dtype=np.float32), -1).astype(
        ml_dtypes.bfloat16)

    in_maps = []
    for c in range(N_CORES):
        b, g = c // GROUPS, c % GROUPS
        cols = _col_index(g)
        wq = np.ascontiguousarray(qws[:, cols])
        wk = np.ascontiguousarray(kw[:, cols]).astype(np.float32)
        qh, ql8, qh8 = _split16(wq, SWQ)
        kh, kl8, kh8 = _split16(wk, SWK)
        wqdr = np.empty((D, 2, CPC), dtype=ml_dtypes.float8_e4m3)
        wqdr[:, 0, :] = ql8
        wqdr[:, 1, :] = qh8
        wkdr = np.empty((D, 2, CPC), dtype=ml_dtypes.float8_e4m3)
        wkdr[:, 0, :] = kl8
        wkdr[:, 1, :] = kh8
        xh, xdr = xsplits[b]
        in_maps.append({
            "xh16": xh,
            "xdr": xdr,
            "wqh": qh,
            "wqdr": wqdr,
            "wkh": kh,
            "wkdr": wkdr,
            "wvh": (np.ascontiguousarray(vw[:, cols]) / SX).astype(
                np.float16),
            "woh": np.ascontiguousarray(
                ow_perm[:, g * CPC:(g + 1) * CPC]).astype(np.float16),
            "mskst": mskst,
            "mskmv": mskmv,
        })
    return in_maps


def assemble_output(results, s=S):
    out = np.empty((B, s, D), dtype=np.float32)
    for c in range(N_CORES):
        b, g = c // GROUPS, c % GROUPS
        oT = results[c]["outT"]  # [NCC, P, s]
        for occ in range(NCC):
            out[b, :, g * CPC + occ * P:(g * CPC + (occ + 1) * P)] = oT[occ].T
    return out


def run_on_hw(x, qw, kw, vw, ow, trace=False, **cfg_over):
    cfg = dict(DEFAULT_CFG)
    cfg.update(cfg_over)
    s = cfg["s"]
    nc = get_nc(**cfg)
    in_maps = make_in_maps(x, qw, kw, vw, ow, s=s)
    res = run_bass_kernel_spmd(nc, in_maps, core_ids=list(range(N_CORES)),
                               trace=trace)
    return assemble_output(res.results, s=s), res


def kernel(x, qw, kw, vw, ow):
    out, _ = run_on_hw(np.asarray(x, dtype=np.float32),
                       np.asarray(qw, dtype=np.float32),
                       np.asarray(kw, dtype=np.float32),
                       np.asarray(vw, dtype=np.float32),
                       np.asarray(ow, dtype=np.float32))
    return out


# revision 23
# speedup vs baseline: 1.2454x; 1.0027x over previous
"""Multi-head attention (dense_transformer) on 8 TRN2 NeuronCores.

Sharding: 2-way data parallel over batch x 4-way tensor parallel over heads.
Core c handles batch b=c//4 and heads {4g..4g+3} where g=c%4 (4 heads, 256
channels per core; channels of head h are qw columns {hd*16+h}).

Architecture v3 ("fp16 + fp8-DoubleRow pseudo-fp32"):
  The pseudo-fp32 matmuls (Q/K projections and K^T Q scores) need ~15-bit
  operand mantissas because score std is ~256 and softmax is near-argmax.
  v2 used 3-term bf16 splits (3 passes).  v3 uses:
    main term:  fp16 x fp16 (11-bit mantissas), 1.0 PE cycles/column
    both cross terms: one fp8-e4m3 matmul in DoubleRow perf mode (two
      stationary/moving stream pairs summed into one PSUM output) at 0.5
      cycles/column.
  => 1.5 pass-equivalents instead of 3.  Per-stream power-of-2 scales keep
  every operand inside e4m3/fp16 range; all passes of one accumulation
  produce the same product scale, which is folded into the EXP scale.

  Score-side operand scales (score PSUM = 128*score; every e4m3 operand
  stays under 224 so the e4m3/e4m3fn variant ambiguity is moot):
    KH16=fp16(4K), KL8=e4m3(4(K-Kh)), KH8=e4m3(Kh)
    QH16=fp16(32Q), QL8=e4m3(128(Q-Qh)), QH8=e4m3(32Qh)
    DR streams: (KL8,QH8) + (KH8,QL8) -> 128*(Kl Qh + Kh Ql)
  Projection pass scales: QPSUM=32768*Q (wq folded 1/32), KPSUM=1024*K;
  both drain with the same ActCopy scale 1/512.

  x and weight hi/lo splits are precomputed on the HOST (no on-device
  split work; x ships as fp16 + e4m3 pair, same bytes as one f32 copy).

  Causal mask is applied on the PE: an extra rank-structured matmul term
  (stationary -1e10*I bf16, moving strictly-lower-triangular ones) adds
  -1e10 to masked elements of the diagonal chunks inside PSUM, before any
  drain -- no DVE masking.

  V projection: single fp16 pass (xh * fp16(wv/16)); V, exp weights, OT,
  and the out-projection all run in fp16 (11-bit) instead of bf16, which
  *reduces* error vs v2 while the out-projection drops to a single pass.

  Softmax sum reciprocal: vector-engine reciprocal_approx_fast (1 DVE op)
  instead of Ln+Exp on ACT (kills the activation-table thrash).

  Engine balance per score chunk: PSUM drain (ACT Copy), running max
  (DVE), subtract (DVE), EXP (ACT).
"""
import sys

sys.path.insert(0, "/opt/trn_rl_repo")

import numpy as np
import ml_dtypes

import concourse.bass as bass
import concourse.mybir as mybir
import concourse.tile as tile
from concourse import bacc
from concourse import bass_isa
from concourse.bass_utils import run_bass_kernel_spmd

# ---- problem constants (hardcoded per harness contract) ----
B, S, D, HEADS = 2, 2048, 1024, 16
N_CORES = 8
GROUPS = 4                 # head-groups == cores per batch
HPC = HEADS // GROUPS      # heads per core (4)
HD = D // HEADS            # 64
CPC = HPC * HD             # channels per core (256)
P = 128
NCC = CPC // P             # col chunks per core (2)
DCH = D // P               # contraction chunks (8)
QB = 512                   # q block width (1 PSUM bank of f32)

f32 = mybir.dt.float32
bf16 = mybir.dt.bfloat16
fp16 = mybir.dt.float16
e4m3 = mybir.dt.float8e4

AX = mybir.AxisListType
EXP = mybir.ActivationFunctionType.Exp
MAXOP = mybir.AluOpType.max
SUB = mybir.AluOpType.subtract
MULT = mybir.AluOpType.mult
COPYF = mybir.ActivationFunctionType.Copy
DR = mybir.MatmulPerfMode.DoubleRow

DEFAULT_CFG = dict(s=S)

# host-side split scales
SWQ = 2048.0   # wq-tilde (=qw/32) fp16-hi scale
SWK = 64.0     # kw fp16-hi scale
SX = 16.0      # x fp16-hi scale
SXL = 256.0    # x e4m3-lo scale
DRAINQ = 1.0 / 1024.0  # QPSUM(32768Q) -> fp16(32Q)
DRAINK = 1.0 / 256.0   # KPSUM(1024K) -> fp16(4K)
SSCORE = 128.0         # score PSUM scale


def build_nc(s=S, dbg=False):
    assert s % QB == 0
    NQB = s // QB            # 512-wide q blocks
    NKC = s // P             # 128-wide k chunks
    KPB = QB // P            # k chunks per q block on the diagonal (4)
    VW = 65                  # V channels per (hp,h2) incl the ones column
    NH2 = NCC * 2            # head slots per core (4)

    nc = bacc.Bacc("TRN2", target_bir_lowering=False, debug=False,
                   num_devices=N_CORES)
    xh16d = nc.dram_tensor("xh16", [D, s], fp16, kind="ExternalInput").ap()
    xdrd = nc.dram_tensor("xdr", [D, 2, s], e4m3, kind="ExternalInput").ap()
    wqh = nc.dram_tensor("wqh", [D, CPC], fp16, kind="ExternalInput").ap()
    wqdr = nc.dram_tensor("wqdr", [D, 2, CPC], e4m3, kind="ExternalInput").ap()
    wkh = nc.dram_tensor("wkh", [D, CPC], fp16, kind="ExternalInput").ap()
    wkdr = nc.dram_tensor("wkdr", [D, 2, CPC], e4m3, kind="ExternalInput").ap()
    wvh = nc.dram_tensor("wvh", [D, CPC], fp16, kind="ExternalInput").ap()
    woh = nc.dram_tensor("woh", [D, CPC], fp16, kind="ExternalInput").ap()
    mskst = nc.dram_tensor("mskst", [P, P], bf16, kind="ExternalInput").ap()
    mskmv = nc.dram_tensor("mskmv", [P, P], bf16, kind="ExternalInput").ap()
    outT = nc.dram_tensor("outT", [NCC, P, s], f32, kind="ExternalOutput").ap()

    with tile.TileContext(nc) as tc:
        with (
            tc.tile_pool(name="cpool", bufs=1) as cpool,
            tc.tile_pool(name="wpool", bufs=1) as wpool,
            tc.tile_pool(name="big", bufs=1) as big,
            tc.tile_pool(name="stat", bufs=3) as stat,
            tc.tile_pool(name="ms", bufs=6) as ms,
            tc.tile_pool(name="op", bufs=2) as op,
            tc.tile_pool(name="dram", bufs=1, space="DRAM") as dpool,
        ):
            # per-(qb,hp) gather buffers; last qb additionally split by h2
            ag_in = {}
            ag_out = {}
            for qb in range(NQB):
                for hp in range(NCC):
                    if qb == NQB - 1:
                        for h2 in range(2):
                            ag_in[qb, hp, h2] = dpool.tile(
                                [64, QB], fp16, tag=f"agi{qb}_{hp}_{h2}",
                                name=f"agi{qb}_{hp}_{h2}")
                            ag_out[qb, hp, h2] = dpool.tile(
                                [GROUPS, 64, QB], fp16,
                                tag=f"ago{qb}_{hp}_{h2}",
                                name=f"ago{qb}_{hp}_{h2}")
                    else:
                        ag_in[qb, hp] = dpool.tile(
                            [P, QB], fp16, tag=f"agi{qb}_{hp}",
                            name=f"agi{qb}_{hp}")
                        ag_out[qb, hp] = dpool.tile(
                            [GROUPS, P, QB], fp16, tag=f"ago{qb}_{hp}",
                            name=f"ago{qb}_{hp}")

            # weights + constants in SBUF
            msk_st = cpool.tile([P, P], bf16, tag="mskst")
            msk_mv = cpool.tile([P, P], bf16, tag="mskmv")
            wqh_sb = wpool.tile([P, DCH, CPC], fp16, tag="wqh")
            wqdr_sb = wpool.tile([P, DCH, 2, CPC], e4m3, tag="wqdr")
            wkh_sb = wpool.tile([P, DCH, CPC], fp16, tag="wkh")
            wkdr_sb = wpool.tile([P, DCH, 2, CPC], e4m3, tag="wkdr")
            wvh_sb = wpool.tile([P, DCH, CPC], fp16, tag="wvh")
            woh_sb = wpool.tile([P, DCH, CPC], fp16, tag="woh")
            # one DMA per weight tensor ([D,...] viewed as [P, DCH, ...]);
            # wo deferred until after qb0's x tiles (not needed in phase 1)
            nc.sync.dma_start(
                wqh_sb[:], wqh.rearrange("(dc p) c -> p dc c", p=P))
            nc.sync.dma_start(
                wqdr_sb[:], wqdr.rearrange("(dc p) t c -> p dc t c", p=P))
            nc.sync.dma_start(
                wkh_sb[:], wkh.rearrange("(dc p) c -> p dc c", p=P))
            nc.sync.dma_start(
                wkdr_sb[:], wkdr.rearrange("(dc p) t c -> p dc t c", p=P))
            nc.sync.dma_start(msk_st[:], mskst)
            nc.sync.dma_start(msk_mv[:], mskmv)
            nc.sync.dma_start(
                wvh_sb[:], wvh.rearrange("(dc p) c -> p dc c", p=P))

            QH16 = big.tile([P, NCC, s], fp16, tag="QH16")
            QDR = big.tile([P, NCC, 2, s], e4m3, tag="QDR")
            KH16 = big.tile([P, NCC, s], fp16, tag="KH16")
            KDR = big.tile([P, NCC, 2, s], e4m3, tag="KDR")
            # Vhat[k, :]: 4 groups of 65 cols: 64 V channels + a ones col
            Vsb = big.tile([P, NKC, NH2 * VW], fp16, tag="Vsb")
            OT = big.tile([P, NCC, s], fp16, tag="OT")

            for g in range(NH2):
                nc.gpsimd.memset(Vsb[:, :, g * VW + 64], 1.0)
            mbias = cpool.tile([P, 1], f32, tag="mbias")
            nc.gpsimd.memset(mbias[:], -6.0)

            # ---------------- phase 1: projections ----------------
            with (
                tc.tile_pool(name="psq", bufs=1, space="PSUM") as psq,
                tc.tile_pool(name="psv", bufs=1, space="PSUM") as psv,
                tc.tile_pool(name="xs", bufs=2) as xs,
                tc.tile_pool(name="xtmp", bufs=2) as xtmp,
            ):
                for qb in range(NQB):
                    qsl = slice(qb * QB, (qb + 1) * QB)
                    xh = []
                    xdr = []
                    for di in range(DCH):
                        dsl = slice(di * P, (di + 1) * P)
                        xht = xs.tile([P, QB], fp16, tag=f"xh{di}",
                                      name=f"xh{di}")
                        nc.sync.dma_start(xht[:], xh16d[dsl, qsl])
                        xdt = xs.tile([P, 2, QB], e4m3, tag=f"xd{di}",
                                      name=f"xd{di}")
                        nc.sync.dma_start(xdt[:], xdrd[dsl, :, qsl])
                        xh.append(xht)
                        xdr.append(xdt)
                    if qb == 0:
                        nc.sync.dma_start(
                            woh_sb[:], woh.rearrange("(dc p) c -> p dc c",
                                                     p=P))
                    for nm, whs, wds, hi_t, dr_t in (
                            ("q", wqh_sb, wqdr_sb, QH16, QDR),
                            ("k", wkh_sb, wkdr_sb, KH16, KDR)):
                        dscale = DRAINQ if nm == "q" else DRAINK
                        for cc in range(NCC):
                            csl = slice(cc * P, (cc + 1) * P)
                            acc = psq.tile([P, QB], f32, tag=f"a{nm}{cc}",
                                           name=f"a{nm}{cc}")
                            for di in range(DCH):
                                nc.tensor.matmul(
                                    acc[:], whs[:, di, csl], xh[di][:],
                                    start=(di == 0), stop=False)
                                nc.tensor.matmul(
                                    acc[:], wds[:, di, :, csl], xdr[di][:],
                                    start=False, stop=(di == DCH - 1),
                                    perf_mode=DR)
                            nc.scalar.activation(hi_t[:, cc, qsl], acc[:],
                                                 COPYF, scale=dscale)
                            if nm == "q":
                                # QL8 = e4m3(128 Ql): STT gives 32Ql in f32,
                                # then ACT rescales x4 into e4m3
                                t32 = xtmp.tile([P, QB], f32, tag="t32",
                                                name="t32")
                                nc.vector.scalar_tensor_tensor(
                                    t32[:], acc[:], dscale,
                                    hi_t[:, cc, qsl], MULT, SUB)
                                nc.scalar.activation(
                                    dr_t[:, cc, 1, qsl], t32[:], COPYF,
                                    scale=4.0)
                                # QH8 = e4m3(32 Qh): plain cast of QH16
                                nc.vector.tensor_copy(
                                    dr_t[:, cc, 0, qsl], hi_t[:, cc, qsl])
                            else:
                                # KL8 = e4m3(4 Kl)
                                nc.vector.scalar_tensor_tensor(
                                    dr_t[:, cc, 0, qsl], acc[:], dscale,
                                    hi_t[:, cc, qsl], MULT, SUB)
                                # KH8 = e4m3(Kh) = KH16 x 1/4
                                nc.scalar.activation(
                                    dr_t[:, cc, 1, qsl], hi_t[:, cc, qsl],
                                    COPYF, scale=0.25)
                    for r in range(KPB):
                        vacc = psv.tile([P, CPC], f32, tag=f"pv{r}",
                                        name=f"pv{r}")
                        for di in range(DCH):
                            nc.tensor.matmul(
                                vacc[:], xh[di][:, r * P:(r + 1) * P],
                                wvh_sb[:, di, :],
                                start=(di == 0), stop=(di == DCH - 1))
                        ki = qb * KPB + r
                        dst = Vsb[:, ki].rearrange(
                            "p (g w) -> p g w", w=VW)[:, :, 0:64]
                        src = vacc[:].rearrange("p (g w) -> p g w", w=64)
                        nc.scalar.activation(dst, src, COPYF)

            # ---------------- phase 2 + 3, software-pipelined ----------------
            with (
                tc.tile_pool(name="pssc", bufs=4, space="PSUM") as pssc,
                tc.tile_pool(name="psot", bufs=2, space="PSUM") as psot,
                tc.tile_pool(name="pso", bufs=1, space="PSUM") as pso,
                tc.tile_pool(name="stgp", bufs=34) as stgp,
                tc.tile_pool(name="expp", bufs=8) as expp,
            ):
                p3_accs = {}

                def phase3_block(j, part=None):
                    """out-proj for q block j (consumes that block's gather).
                    part=0/1 emits only the first/second half of the
                    contraction (used to overlap the last block's tail)."""
                    qsl3 = slice(j * QB, (j + 1) * QB)
                    last = (j == NQB - 1)
                    if part != 1:
                        p3_accs[j] = [pso.tile([P, QB], f32, tag=f"po{occ}",
                                               name=f"po{occ}")
                                      for occ in range(NCC)]
                    accs = p3_accs[j]
                    # cc-major order so the last block's late gathers (hp=1)
                    # are needed as late as possible
                    order = (sorted(range(DCH), key=lambda m: (m % NCC, m // NCC))
                             if last else list(range(DCH)))
                    rng = {None: range(DCH), 0: range(DCH // 2),
                           1: range(DCH // 2, DCH)}[part]
                    for i in rng:
                        mch = order[i]
                        g_, cc_ = mch // NCC, mch % NCC
                        mt = ms.tile([P, QB], fp16, tag="mt", name="mt")
                        if last:
                            nc.sync.dma_start(mt[0:64, :],
                                              ag_out[j, cc_, 0][g_])
                            nc.sync.dma_start(mt[64:128, :],
                                              ag_out[j, cc_, 1][g_])
                        else:
                            nc.sync.dma_start(mt[:], ag_out[j, cc_][g_])
                        for occ in range(NCC):
                            nc.tensor.matmul(
                                accs[occ][:],
                                woh_sb[:, mch, occ * P:(occ + 1) * P],
                                mt[:], start=(i == 0), stop=(i == DCH - 1))
                    if part == 0:
                        return
                    for occ in range(NCC):
                        oo = op.tile([P, QB], f32, tag="oo", name="oo")
                        nc.any.tensor_copy(oo[:], accs[occ][:])
                        nc.sync.dma_start(outT[occ, :, qsl3], oo[:])

                def _gather(inp, outp):
                    nc.gpsimd.collective_compute(
                        "AllGather", mybir.AluOpType.bypass,
                        replica_groups=[[0, 1, 2, 3], [4, 5, 6, 7]],
                        ins=[inp], outs=[outp],
                    )

                def emit_A_chunk(st, kc):
                    qb, hp, h2, sts, rm, _ = st
                    hsl = slice(h2 * 64, (h2 + 1) * 64)
                    diag = kc - qb * KPB
                    off = max(0, diag) * P
                    psc = pssc.tile([P, QB], f32, tag="psc", name="psc")
                    ksl = slice(kc * P, (kc + 1) * P)
                    mvsl = slice(qb * QB + off, (qb + 1) * QB)
                    nc.tensor.matmul(
                        psc[:, off:], KH16[hsl, hp, ksl],
                        QH16[hsl, hp, mvsl], start=True, stop=False)
                    nc.tensor.matmul(
                        psc[:, off:], KDR[hsl, hp, :, ksl],
                        QDR[hsl, hp, :, mvsl], start=False,
                        stop=(diag < 0), perf_mode=DR)
                    if diag >= 0:
                        nc.tensor.matmul(
                            psc[:, off:off + P], msk_st[:],
                            msk_mv[:], start=False, stop=True)
                    stg = stgp.tile([P, QB], f32, tag="stg", name="stg")
                    nc.scalar.activation(stg[:, off:], psc[:, off:], COPYF)
                    if kc == 0:
                        nc.vector.tensor_copy(rm[:], stg[:])
                    else:
                        nc.vector.tensor_tensor(rm[:, off:], rm[:, off:],
                                                stg[:, off:], MAXOP)
                    sts.append((stg, off))

                def emit_B_chunk(st, kc):
                    qb, hp, h2, sts, _, ctx = st
                    otp, mrep = ctx
                    vg = slice((hp * 2 + h2) * VW, (hp * 2 + h2 + 1) * VW)
                    nkc = len(sts)
                    stg, off = sts[kc]
                    nc.vector.tensor_tensor(stg[:, off:], stg[:, off:],
                                            mrep[:, off:], SUB)
                    ex = expp.tile([P, QB], fp16, tag="ex", name="ex")
                    nc.scalar.activation(ex[:, off:], stg[:, off:], EXP,
                                         bias=mbias[:], scale=1.0 / SSCORE)
                    nc.tensor.matmul(otp[:, off:], Vsb[:, kc, vg],
                                     ex[:, off:],
                                     start=(kc == 0), stop=(kc == nkc - 1))

                def finish_B(st):
                    qb, hp, h2, sts, _, ctx = st
                    otp, mrep = ctx
                    qsl = slice(qb * QB, (qb + 1) * QB)
                    hsl = slice(h2 * 64, (h2 + 1) * 64)
                    lsb = stat.tile([1, QB], f32, tag="lsb", name="lsb")
                    nc.vector.tensor_copy(lsb[:], otp[64:65, :])
                    rec = stat.tile([1, QB], f32, tag="rec", name="rec")
                    nc.vector.reciprocal_approx_fast(rec[:], lsb[:])
                    recb = stat.tile([64, QB], f32, tag="recb", name="recb")
                    nc.gpsimd.partition_broadcast(recb[:], rec[:], 64)
                    nc.vector.tensor_tensor(OT[hsl, hp, qsl], otp[0:64, :],
                                            recb[:], MULT)

                def fire_gather(st):
                    """gather trigger + out-proj injection, deferred so the
                    collective trigger's gpsimd-queue stall lands after the
                    next block's partition_all_reduce."""
                    qb, hp, h2, _, _, _ = st
                    qsl = slice(qb * QB, (qb + 1) * QB)
                    hsl = slice(h2 * 64, (h2 + 1) * 64)
                    if qb == NQB - 1:
                        nc.sync.dma_start(ag_in[qb, hp, h2][:],
                                          OT[hsl, hp, qsl])
                        _gather(ag_in[qb, hp, h2][:], ag_out[qb, hp, h2][:])
                        if hp == 0 and h2 == 1 and qb >= 1:
                            phase3_block(qb - 1)
                    elif h2 == 1:
                        nc.sync.dma_start(ag_in[qb, hp][:], OT[:, hp, qsl])
                        _gather(ag_in[qb, hp][:], ag_out[qb, hp][:])
                        if hp == NCC - 1 and qb >= 1:
                            phase3_block(qb - 1)

                LAG = 8   # A-chunks between a block's reduce and its first AV

                def reduce_block(cur):
                    mrep = stat.tile([P, QB], bf16, tag="mrep", name="mrep")
                    nc.gpsimd.partition_all_reduce(
                        mrep[:], cur[4][:], P, bass_isa.ReduceOp.max)
                    otp = psot.tile([VW, QB], f32, tag="otp", name="otp")
                    return cur[:5] + ((otp, mrep),)

                blocks = [(qb, hp, h2) for qb in range(NQB)
                          for hp in range(NCC) for h2 in range(2)]
                prev = None
                pend0 = []   # depth-3 pipeline for the small first-qb blocks
                for qb, hp, h2 in blocks:
                    rm = stat.tile([P, QB], bf16, tag="rm", name="rm")
                    cur = (qb, hp, h2, [], rm, None)
                    nA = qb * KPB + KPB
                    if qb == 0:
                        for i in range(nA):
                            emit_A_chunk(cur, i)
                        done = None
                        if len(pend0) >= 2:
                            done = pend0.pop(0)
                            for i in range(len(done[3])):
                                emit_B_chunk(done, i)
                            finish_B(done)
                        pend0.append(reduce_block(cur))
                        if done is not None:
                            fire_gather(done)
                        continue
                    if pend0:
                        # drain the depth-3 queue down to one carried block
                        while len(pend0) > 1:
                            done = pend0.pop(0)
                            for i in range(len(done[3])):
                                emit_B_chunk(done, i)
                            finish_B(done)
                            fire_gather(done)
                        prev = pend0.pop(0)
                    nB = len(prev[3]) if prev else 0
                    for i in range(max(nA, nB + LAG)):
                        if i < nA:
                            emit_A_chunk(cur, i)
                        j = i - LAG
                        if prev is not None and 0 <= j < nB:
                            emit_B_chunk(prev, j)
                    if prev is not None:
                        finish_B(prev)
                    cur = reduce_block(cur)
                    if prev is not None:
                        fire_gather(prev)
                    prev = cur
                # flush any depth-3 remainder (only when NQB == 1)
                while len(pend0) > 1:
                    done = pend0.pop(0)
                    for i in range(len(done[3])):
                        emit_B_chunk(done, i)
                    finish_B(done)
                    fire_gather(done)
                if pend0:
                    prev = pend0.pop(0)
                # tail: final block's B, with the gather-independent half of
                # the last out-proj interleaved in
                nB = len(prev[3])
                for i in range(nB):
                    emit_B_chunk(prev, i)
                    if i == 3:
                        phase3_block(NQB - 1, part=0)
                finish_B(prev)
                fire_gather(prev)
                phase3_block(NQB - 1, part=1)

    nc.compile()
    return nc


_NC_CACHE = {}


def get_nc(**cfg):
    key = tuple(sorted(cfg.items()))
    if key not in _NC_CACHE:
        _NC_CACHE[key] = build_nc(**cfg)
    return _NC_CACHE[key]


def _col_index(g):
    p = np.arange(CPC)
    return (p % HD) * HEADS + (HPC * g + p // HD)


def _ow_row_index():
    r = np.arange(D)
    m, p128 = r // P, r % P
    g_, cc = m // NCC, m % NCC
    p256 = cc * P + p128
    lh, hd = p256 // HD, p256 % HD
    return hd * HEADS + (HPC * g_ + lh)


def _split16(w, shi):
    """w -> (fp16(shi*w), e4m3(shi*(w-hi/shi)), e4m3(shi/16*whi))"""
    hi = (shi * w).astype(np.float16)
    lo = shi * w - hi.astype(np.float32)
    l8 = lo.astype(ml_dtypes.float8_e4m3)
    h8 = (hi.astype(np.float32) / 16.0).astype(ml_dtypes.float8_e4m3)
    return hi, l8, h8


def make_in_maps(x, qw, kw, vw, ow, s=S):
    scale = 1.0 / np.sqrt(np.float32(D))
    qws = (qw * scale).astype(np.float32)
    ow_perm = np.ascontiguousarray(ow[_ow_row_index()])

    # x hi/lo splits, shared per batch
    xsplits = []
    for b in range(B):
        xT = np.ascontiguousarray(x[b, :s].T).astype(np.float32)
        xh = (SX * xT).astype(np.float16)          # fp16(16 x)
        xl = SX * xT - xh.astype(np.float32)       # 16 xl
        xdr = np.empty((D, 2, s), dtype=ml_dtypes.float8_e4m3)
        xdr[:, 0, :] = xh.astype(ml_dtypes.float8_e4m3)   # e4m3(16 xh)
        xdr[:, 1, :] = (SXL / SX * xl).astype(ml_dtypes.float8_e4m3)
        xsplits.append((xh, xdr))

    mskst = (-1e10 * np.eye(P, dtype=np.float32)).astype(ml_dtypes.bfloat16)
    mskmv = np.tril(np.ones((P, P), dtype=np.float32), -1).astype(
        ml_dtypes.bfloat16)

    in_maps = []
    for c in range(N_CORES):
        b, g = c // GROUPS, c % GROUPS
        cols = _col_index(g)
        wq = np.ascontiguousarray(qws[:, cols])
        wk = np.ascontiguousarray(kw[:, cols]).astype(np.float32)
        qh, ql8, qh8 = _split16(wq, SWQ)
        kh, kl8, kh8 = _split16(wk, SWK)
        wqdr = np.empty((D, 2, CPC), dtype=ml_dtypes.float8_e4m3)
        wqdr[:, 0, :] = ql8
        wqdr[:, 1, :] = qh8
        wkdr = np.empty((D, 2, CPC), dtype=ml_dtypes.float8_e4m3)
        wkdr[:, 0, :] = kl8
        wkdr[:, 1, :] = kh8
        xh, xdr = xsplits[b]
        in_maps.append({
            "xh16": xh,
            "xdr": xdr,
            "wqh": qh,
            "wqdr": wqdr,
            "wkh": kh,
            "wkdr": wkdr,
            "wvh": (np.ascontiguousarray(vw[:, cols]) / SX).astype(
                np.float16),
            "woh": np.ascontiguousarray(
                ow_perm[:, g * CPC:(g + 1) * CPC]).astype(np.float16),
            "mskst": mskst,
            "mskmv": mskmv,
        })
    return in_maps


def assemble_output(results, s=S):
    out = np.empty((B, s, D), dtype=np.float32)
    for c in range(N_CORES):
        b, g = c // GROUPS, c % GROUPS
        oT = results[c]["outT"]  # [NCC, P, s]
        for occ in range(NCC):
            out[b, :, g * CPC + occ * P:(g * CPC + (occ + 1) * P)] = oT[occ].T
    return out


def run_on_hw(x, qw, kw, vw, ow, trace=False, **cfg_over):
    cfg = dict(DEFAULT_CFG)
    cfg.update(cfg_over)
    s = cfg["s"]
    nc = get_nc(**cfg)
    in_maps = make_in_maps(x, qw, kw, vw, ow, s=s)
    res = run_bass_kernel_spmd(nc, in_maps, core_ids=list(range(N_CORES)),
                               trace=trace)
    return assemble_output(res.results, s=s), res


def kernel(x, qw, kw, vw, ow):
    out, _ = run_on_hw(np.asarray(x, dtype=np.float32),
                       np.asarray(qw, dtype=np.float32),
                       np.asarray(kw, dtype=np.float32),
                       np.asarray(vw, dtype=np.float32),
                       np.asarray(ow, dtype=np.float32))
    return out


# revision 28
# speedup vs baseline: 1.2747x; 1.0236x over previous
"""Multi-head attention (dense_transformer) on 8 TRN2 NeuronCores.

Sharding: 2-way data parallel over batch x 4-way tensor parallel over heads.
Core c handles batch b=c//4 and heads {4g..4g+3} where g=c%4 (4 heads, 256
channels per core; channels of head h are qw columns {hd*16+h}).

Architecture v3 ("fp16 + fp8-DoubleRow pseudo-fp32"):
  The pseudo-fp32 matmuls (Q/K projections and K^T Q scores) need ~15-bit
  operand mantissas because score std is ~256 and softmax is near-argmax.
  v2 used 3-term bf16 splits (3 passes).  v3 uses:
    main term:  fp16 x fp16 (11-bit mantissas), 1.0 PE cycles/column
    both cross terms: one fp8-e4m3 matmul in DoubleRow perf mode (two
      stationary/moving stream pairs summed into one PSUM output) at 0.5
      cycles/column.
  => 1.5 pass-equivalents instead of 3.  Per-stream power-of-2 scales keep
  every operand inside e4m3/fp16 range; all passes of one accumulation
  produce the same product scale, which is folded into the EXP scale.

  Score-side operand scales (score PSUM = 128*score; every e4m3 operand
  stays under 224 so the e4m3/e4m3fn variant ambiguity is moot):
    KH16=fp16(4K), KL8=e4m3(4(K-Kh)), KH8=e4m3(Kh)
    QH16=fp16(32Q), QL8=e4m3(128(Q-Qh)), QH8=e4m3(32Qh)
    DR streams: (KL8,QH8) + (KH8,QL8) -> 128*(Kl Qh + Kh Ql)
  Projection pass scales: QPSUM=32768*Q (wq folded 1/32), KPSUM=1024*K;
  both drain with the same ActCopy scale 1/512.

  x and weight hi/lo splits are precomputed on the HOST (no on-device
  split work; x ships as fp16 + e4m3 pair, same bytes as one f32 copy).

  Causal mask is applied on the PE: an extra rank-structured matmul term
  (stationary -1e10*I bf16, moving strictly-lower-triangular ones) adds
  -1e10 to masked elements of the diagonal chunks inside PSUM, before any
  drain -- no DVE masking.

  V projection: single fp16 pass (xh * fp16(wv/16)); V, exp weights, OT,
  and the out-projection all run in fp16 (11-bit) instead of bf16, which
  *reduces* error vs v2 while the out-projection drops to a single pass.

  Softmax sum reciprocal: vector-engine reciprocal_approx_fast (1 DVE op)
  instead of Ln+Exp on ACT (kills the activation-table thrash).

  Engine balance per score chunk: PSUM drain (ACT Copy), running max
  (DVE), subtract (DVE), EXP (ACT).
"""
import sys

sys.path.insert(0, "/opt/trn_rl_repo")

import numpy as np
import ml_dtypes

import concourse.bass as bass
import concourse.mybir as mybir
import concourse.tile as tile
from concourse import bacc
from concourse import bass_isa
from concourse.bass_utils import run_bass_kernel_spmd

# ---- problem constants (hardcoded per harness contract) ----
B, S, D, HEADS = 2, 2048, 1024, 16
N_CORES = 8
GROUPS = 4                 # head-groups == cores per batch
HPC = HEADS // GROUPS      # heads per core (4)
HD = D // HEADS            # 64
CPC = HPC * HD             # channels per core (256)
P = 128
NCC = CPC // P             # col chunks per core (2)
DCH = D // P               # contraction chunks (8)
QB = 512                   # q block width (1 PSUM bank of f32)

f32 = mybir.dt.float32
bf16 = mybir.dt.bfloat16
fp16 = mybir.dt.float16
e4m3 = mybir.dt.float8e4

AX = mybir.AxisListType
EXP = mybir.ActivationFunctionType.Exp
MAXOP = mybir.AluOpType.max
SUB = mybir.AluOpType.subtract
MULT = mybir.AluOpType.mult
COPYF = mybir.ActivationFunctionType.Copy
DR = mybir.MatmulPerfMode.DoubleRow

DEFAULT_CFG = dict(s=S)

# host-side split scales
SWQ = 2048.0   # wq-tilde (=qw/32) fp16-hi scale
SWK = 64.0     # kw fp16-hi scale
SX = 16.0      # x fp16-hi scale
SXL = 256.0    # x e4m3-lo scale
DRAINQ = 1.0 / 1024.0  # QPSUM(32768Q) -> fp16(32Q)
DRAINK = 1.0 / 256.0   # KPSUM(1024K) -> fp16(4K)
SSCORE = 128.0         # score PSUM scale


def build_nc(s=S, dbg=False):
    assert s % QB == 0
    NQB = s // QB            # 512-wide q blocks
    NKC = s // P             # 128-wide k chunks
    KPB = QB // P            # k chunks per q block on the diagonal (4)
    VW = 65                  # V channels per (hp,h2) incl the ones column
    NH2 = NCC * 2            # head slots per core (4)

    nc = bacc.Bacc("TRN2", target_bir_lowering=False, debug=False,
                   num_devices=N_CORES)
    xh16d = nc.dram_tensor("xh16", [D, s], fp16, kind="ExternalInput").ap()
    xdrd = nc.dram_tensor("xdr", [D, 2, s], e4m3, kind="ExternalInput").ap()
    wqh = nc.dram_tensor("wqh", [D, CPC], fp16, kind="ExternalInput").ap()
    wqdr = nc.dram_tensor("wqdr", [D, 2, CPC], e4m3, kind="ExternalInput").ap()
    wkh = nc.dram_tensor("wkh", [D, CPC], fp16, kind="ExternalInput").ap()
    wkdr = nc.dram_tensor("wkdr", [D, 2, CPC], e4m3, kind="ExternalInput").ap()
    wvh = nc.dram_tensor("wvh", [D, CPC], fp16, kind="ExternalInput").ap()
    woh = nc.dram_tensor("woh", [D, CPC], fp16, kind="ExternalInput").ap()
    mskst = nc.dram_tensor("mskst", [P, P], bf16, kind="ExternalInput").ap()
    mskmv = nc.dram_tensor("mskmv", [P, P], bf16, kind="ExternalInput").ap()
    outT = nc.dram_tensor("outT", [NCC, P, s], f32, kind="ExternalOutput").ap()

    with tile.TileContext(nc) as tc:
        with (
            tc.tile_pool(name="cpool", bufs=1) as cpool,
            tc.tile_pool(name="wpool", bufs=1) as wpool,
            tc.tile_pool(name="big", bufs=1) as big,
            tc.tile_pool(name="stat", bufs=3) as stat,
            tc.tile_pool(name="ms", bufs=6) as ms,
            tc.tile_pool(name="op", bufs=2) as op,
            tc.tile_pool(name="dram", bufs=1, space="DRAM") as dpool,
        ):
            # per-(qb,hp) gather buffers; last qb additionally split by h2
            ag_in = {}
            ag_out = {}
            for qb in range(NQB):
                for hp in range(NCC):
                    if qb == NQB - 1:
                        for h2 in range(2):
                            ag_in[qb, hp, h2] = dpool.tile(
                                [64, QB], fp16, tag=f"agi{qb}_{hp}_{h2}",
                                name=f"agi{qb}_{hp}_{h2}")
                            ag_out[qb, hp, h2] = dpool.tile(
                                [GROUPS, 64, QB], fp16,
                                tag=f"ago{qb}_{hp}_{h2}",
                                name=f"ago{qb}_{hp}_{h2}")
                    else:
                        ag_in[qb, hp] = dpool.tile(
                            [P, QB], fp16, tag=f"agi{qb}_{hp}",
                            name=f"agi{qb}_{hp}")
                        ag_out[qb, hp] = dpool.tile(
                            [GROUPS, P, QB], fp16, tag=f"ago{qb}_{hp}",
                            name=f"ago{qb}_{hp}")

            # weights + constants in SBUF
            msk_st = cpool.tile([P, P], bf16, tag="mskst")
            msk_mv = cpool.tile([P, P], bf16, tag="mskmv")
            wqh_sb = wpool.tile([P, DCH, CPC], fp16, tag="wqh")
            wqdr_sb = wpool.tile([P, DCH, 2, CPC], e4m3, tag="wqdr")
            wkh_sb = wpool.tile([P, DCH, CPC], fp16, tag="wkh")
            wkdr_sb = wpool.tile([P, DCH, 2, CPC], e4m3, tag="wkdr")
            wvh_sb = wpool.tile([P, DCH, CPC], fp16, tag="wvh")
            woh_sb = wpool.tile([P, DCH, CPC], fp16, tag="woh")
            # one DMA per weight tensor ([D,...] viewed as [P, DCH, ...]);
            # wo deferred until after qb0's x tiles (not needed in phase 1)
            nc.sync.dma_start(
                wqh_sb[:], wqh.rearrange("(dc p) c -> p dc c", p=P))
            nc.sync.dma_start(
                wqdr_sb[:], wqdr.rearrange("(dc p) t c -> p dc t c", p=P))
            nc.sync.dma_start(
                wkh_sb[:], wkh.rearrange("(dc p) c -> p dc c", p=P))
            nc.sync.dma_start(
                wkdr_sb[:], wkdr.rearrange("(dc p) t c -> p dc t c", p=P))
            nc.sync.dma_start(msk_st[:], mskst)
            nc.sync.dma_start(msk_mv[:], mskmv)
            nc.sync.dma_start(
                wvh_sb[:], wvh.rearrange("(dc p) c -> p dc c", p=P))

            QH16 = big.tile([P, NCC, s], fp16, tag="QH16")
            QDR = big.tile([P, NCC, 2, s], e4m3, tag="QDR")
            KH16 = big.tile([P, NCC, s], fp16, tag="KH16")
            KDR = big.tile([P, NCC, 2, s], e4m3, tag="KDR")
            # Vhat[k, :]: 4 groups of 65 cols: 64 V channels + a ones col
            Vsb = big.tile([P, NKC, NH2 * VW], fp16, tag="Vsb")
            OT = big.tile([P, NCC, s], fp16, tag="OT")

            for g in range(NH2):
                nc.gpsimd.memset(Vsb[:, :, g * VW + 64], 1.0)
            mbias = cpool.tile([P, 1], f32, tag="mbias")
            nc.gpsimd.memset(mbias[:], -6.0)

            # warm up the collective path with a tiny dummy gather so the
            # first real gather doesn't pay the ~18us channel-warmup
            warm_i = dpool.tile([64, P], fp16, tag="warm_i", name="warm_i")
            warm_o = dpool.tile([GROUPS, 64, P], fp16, tag="warm_o",
                                name="warm_o")
            wz = cpool.tile([64, P], fp16, tag="wz")
            nc.gpsimd.memset(wz[:], 0.0)
            nc.sync.dma_start(warm_i[:], wz[:])
            nc.gpsimd.collective_compute(
                "AllGather", mybir.AluOpType.bypass,
                replica_groups=[[0, 1, 2, 3], [4, 5, 6, 7]],
                ins=[warm_i[:]], outs=[warm_o[:]],
            )

            # ---------------- phase 1: projections ----------------
            with (
                tc.tile_pool(name="psq", bufs=1, space="PSUM") as psq,
                tc.tile_pool(name="psv", bufs=1, space="PSUM") as psv,
                tc.tile_pool(name="xs", bufs=2) as xs,
                tc.tile_pool(name="xtmp", bufs=2) as xtmp,
            ):
                for qb in range(NQB):
                    qsl = slice(qb * QB, (qb + 1) * QB)
                    xh = []
                    xdr = []
                    for di in range(DCH):
                        dsl = slice(di * P, (di + 1) * P)
                        xht = xs.tile([P, QB], fp16, tag=f"xh{di}",
                                      name=f"xh{di}")
                        nc.sync.dma_start(xht[:], xh16d[dsl, qsl])
                        xdt = xs.tile([P, 2, QB], e4m3, tag=f"xd{di}",
                                      name=f"xd{di}")
                        nc.sync.dma_start(xdt[:], xdrd[dsl, :, qsl])
                        xh.append(xht)
                        xdr.append(xdt)
                    if qb == 0:
                        nc.sync.dma_start(
                            woh_sb[:], woh.rearrange("(dc p) c -> p dc c",
                                                     p=P))
                    for nm, whs, wds, hi_t, dr_t in (
                            ("q", wqh_sb, wqdr_sb, QH16, QDR),
                            ("k", wkh_sb, wkdr_sb, KH16, KDR)):
                        dscale = DRAINQ if nm == "q" else DRAINK
                        for cc in range(NCC):
                            csl = slice(cc * P, (cc + 1) * P)
                            acc = psq.tile([P, QB], f32, tag=f"a{nm}{cc}",
                                           name=f"a{nm}{cc}")
                            for di in range(DCH):
                                nc.tensor.matmul(
                                    acc[:], whs[:, di, csl], xh[di][:],
                                    start=(di == 0), stop=False)
                                nc.tensor.matmul(
                                    acc[:], wds[:, di, :, csl], xdr[di][:],
                                    start=False, stop=(di == DCH - 1),
                                    perf_mode=DR)
                            nc.scalar.activation(hi_t[:, cc, qsl], acc[:],
                                                 COPYF, scale=dscale)
                            if nm == "q":
                                # QL8 = e4m3(128 Ql): STT gives 32Ql in f32,
                                # then ACT rescales x4 into e4m3
                                t32 = xtmp.tile([P, QB], f32, tag="t32",
                                                name="t32")
                                nc.vector.scalar_tensor_tensor(
                                    t32[:], acc[:], dscale,
                                    hi_t[:, cc, qsl], MULT, SUB)
                                nc.scalar.activation(
                                    dr_t[:, cc, 1, qsl], t32[:], COPYF,
                                    scale=4.0)
                                # QH8 = e4m3(32 Qh): plain cast of QH16
                                nc.vector.tensor_copy(
                                    dr_t[:, cc, 0, qsl], hi_t[:, cc, qsl])
                            else:
                                # KL8 = e4m3(4 Kl)
                                nc.vector.scalar_tensor_tensor(
                                    dr_t[:, cc, 0, qsl], acc[:], dscale,
                                    hi_t[:, cc, qsl], MULT, SUB)
                                # KH8 = e4m3(Kh) = KH16 x 1/4
                                nc.scalar.activation(
                                    dr_t[:, cc, 1, qsl], hi_t[:, cc, qsl],
                                    COPYF, scale=0.25)
                    for r in range(KPB):
                        vacc = psv.tile([P, CPC], f32, tag=f"pv{r}",
                                        name=f"pv{r}")
                        for di in range(DCH):
                            nc.tensor.matmul(
                                vacc[:], xh[di][:, r * P:(r + 1) * P],
                                wvh_sb[:, di, :],
                                start=(di == 0), stop=(di == DCH - 1))
                        ki = qb * KPB + r
                        dst = Vsb[:, ki].rearrange(
                            "p (g w) -> p g w", w=VW)[:, :, 0:64]
                        src = vacc[:].rearrange("p (g w) -> p g w", w=64)
                        nc.scalar.activation(dst, src, COPYF)

            # ---------------- phase 2 + 3, software-pipelined ----------------
            with (
                tc.tile_pool(name="pssc", bufs=4, space="PSUM") as pssc,
                tc.tile_pool(name="psot", bufs=2, space="PSUM") as psot,
                tc.tile_pool(name="pso", bufs=1, space="PSUM") as pso,
                tc.tile_pool(name="stgp", bufs=34) as stgp,
                tc.tile_pool(name="expp", bufs=8) as expp,
            ):
                p3_accs = {}

                def phase3_block(j, part=None):
                    """out-proj for q block j (consumes that block's gather).
                    part=0/1 emits only the first/second half of the
                    contraction (used to overlap the last block's tail)."""
                    qsl3 = slice(j * QB, (j + 1) * QB)
                    last = (j == NQB - 1)
                    if part != 1:
                        p3_accs[j] = [pso.tile([P, QB], f32, tag=f"po{occ}",
                                               name=f"po{occ}")
                                      for occ in range(NCC)]
                    accs = p3_accs[j]
                    # cc-major order so the last block's late gathers (hp=1)
                    # are needed as late as possible
                    order = (sorted(range(DCH), key=lambda m: (m % NCC, m // NCC))
                             if last else list(range(DCH)))
                    rng = {None: range(DCH), 0: range(DCH // 2),
                           1: range(DCH // 2, DCH)}[part]
                    for i in rng:
                        mch = order[i]
                        g_, cc_ = mch // NCC, mch % NCC
                        mt = ms.tile([P, QB], fp16, tag="mt", name="mt")
                        if last:
                            nc.sync.dma_start(mt[0:64, :],
                                              ag_out[j, cc_, 0][g_])
                            nc.sync.dma_start(mt[64:128, :],
                                              ag_out[j, cc_, 1][g_])
                        else:
                            nc.sync.dma_start(mt[:], ag_out[j, cc_][g_])
                        for occ in range(NCC):
                            nc.tensor.matmul(
                                accs[occ][:],
                                woh_sb[:, mch, occ * P:(occ + 1) * P],
                                mt[:], start=(i == 0), stop=(i == DCH - 1))
                    if part == 0:
                        return
                    for occ in range(NCC):
                        oo = op.tile([P, QB], f32, tag="oo", name="oo")
                        nc.any.tensor_copy(oo[:], accs[occ][:])
                        nc.sync.dma_start(outT[occ, :, qsl3], oo[:])

                def _gather(inp, outp):
                    nc.gpsimd.collective_compute(
                        "AllGather", mybir.AluOpType.bypass,
                        replica_groups=[[0, 1, 2, 3], [4, 5, 6, 7]],
                        ins=[inp], outs=[outp],
                    )

                def emit_A_chunk(st, kc):
                    qb, hp, h2, sts, rm, _ = st
                    hsl = slice(h2 * 64, (h2 + 1) * 64)
                    diag = kc - qb * KPB
                    off = max(0, diag) * P
                    psc = pssc.tile([P, QB], f32, tag="psc", name="psc")
                    ksl = slice(kc * P, (kc + 1) * P)
                    mvsl = slice(qb * QB + off, (qb + 1) * QB)
                    nc.tensor.matmul(
                        psc[:, off:], KH16[hsl, hp, ksl],
                        QH16[hsl, hp, mvsl], start=True, stop=False)
                    nc.tensor.matmul(
                        psc[:, off:], KDR[hsl, hp, :, ksl],
                        QDR[hsl, hp, :, mvsl], start=False,
                        stop=(diag < 0), perf_mode=DR)
                    if diag >= 0:
                        nc.tensor.matmul(
                            psc[:, off:off + P], msk_st[:],
                            msk_mv[:], start=False, stop=True)
                    stg = stgp.tile([P, QB], f32, tag="stg", name="stg")
                    nc.scalar.activation(stg[:, off:], psc[:, off:], COPYF)
                    if kc == 0:
                        nc.vector.tensor_copy(rm[:], stg[:])
                    else:
                        nc.vector.tensor_tensor(rm[:, off:], rm[:, off:],
                                                stg[:, off:], MAXOP)
                    sts.append((stg, off))

                def emit_B_chunk(st, kc):
                    qb, hp, h2, sts, _, ctx = st
                    otp, mrep = ctx
                    vg = slice((hp * 2 + h2) * VW, (hp * 2 + h2 + 1) * VW)
                    nkc = len(sts)
                    stg, off = sts[kc]
                    nc.vector.tensor_tensor(stg[:, off:], stg[:, off:],
                                            mrep[:, off:], SUB)
                    ex = expp.tile([P, QB], fp16, tag="ex", name="ex")
                    nc.scalar.activation(ex[:, off:], stg[:, off:], EXP,
                                         bias=mbias[:], scale=1.0 / SSCORE)
                    nc.tensor.matmul(otp[:, off:], Vsb[:, kc, vg],
                                     ex[:, off:],
                                     start=(kc == 0), stop=(kc == nkc - 1))

                def finish_B(st):
                    qb, hp, h2, sts, _, ctx = st
                    otp, mrep = ctx
                    qsl = slice(qb * QB, (qb + 1) * QB)
                    hsl = slice(h2 * 64, (h2 + 1) * 64)
                    lsb = stat.tile([1, QB], f32, tag="lsb", name="lsb")
                    nc.vector.tensor_copy(lsb[:], otp[64:65, :])
                    rec = stat.tile([1, QB], f32, tag="rec", name="rec")
                    nc.vector.reciprocal_approx_fast(rec[:], lsb[:])
                    recb = stat.tile([64, QB], f32, tag="recb", name="recb")
                    nc.gpsimd.partition_broadcast(recb[:], rec[:], 64)
                    nc.vector.tensor_tensor(OT[hsl, hp, qsl], otp[0:64, :],
                                            recb[:], MULT)

                def fire_gather(st):
                    """gather trigger + out-proj injection, deferred so the
                    collective trigger's gpsimd-queue stall lands after the
                    next block's partition_all_reduce."""
                    qb, hp, h2, _, _, _ = st
                    qsl = slice(qb * QB, (qb + 1) * QB)
                    hsl = slice(h2 * 64, (h2 + 1) * 64)
                    if qb == NQB - 1:
                        nc.sync.dma_start(ag_in[qb, hp, h2][:],
                                          OT[hsl, hp, qsl])
                        _gather(ag_in[qb, hp, h2][:], ag_out[qb, hp, h2][:])
                        if hp == 0 and h2 == 1 and qb >= 3:
                            phase3_block(qb - 2)
                    elif h2 == 1:
                        nc.sync.dma_start(ag_in[qb, hp][:], OT[:, hp, qsl])
                        _gather(ag_in[qb, hp][:], ag_out[qb, hp][:])
                        if hp == NCC - 1 and qb >= 2:
                            phase3_block(qb - 2)

                LAG = 8   # A-chunks between a block's reduce and its first AV

                def reduce_block(cur):
                    mrep = stat.tile([P, QB], bf16, tag="mrep", name="mrep")
                    nc.gpsimd.partition_all_reduce(
                        mrep[:], cur[4][:], P, bass_isa.ReduceOp.max)
                    otp = psot.tile([VW, QB], f32, tag="otp", name="otp")
                    return cur[:5] + ((otp, mrep),)

                blocks = [(qb, hp, h2) for qb in range(NQB)
                          for hp in range(NCC) for h2 in range(2)]
                prev = None
                pend0 = []   # depth-3 pipeline for the small first-qb blocks
                for qb, hp, h2 in blocks:
                    rm = stat.tile([P, QB], bf16, tag="rm", name="rm")
                    cur = (qb, hp, h2, [], rm, None)
                    nA = qb * KPB + KPB
                    if qb == 0:
                        for i in range(nA):
                            emit_A_chunk(cur, i)
                        done = None
                        if len(pend0) >= 2:
                            done = pend0.pop(0)
                            for i in range(len(done[3])):
                                emit_B_chunk(done, i)
                            finish_B(done)
                        pend0.append(reduce_block(cur))
                        if done is not None:
                            fire_gather(done)
                        continue
                    if pend0:
                        # drain the depth-3 queue down to one carried block
                        while len(pend0) > 1:
                            done = pend0.pop(0)
                            for i in range(len(done[3])):
                                emit_B_chunk(done, i)
                            finish_B(done)
                            fire_gather(done)
                        prev = pend0.pop(0)
                    nB = len(prev[3]) if prev else 0
                    for i in range(max(nA, nB + LAG)):
                        if i < nA:
                            emit_A_chunk(cur, i)
                        j = i - LAG
                        if prev is not None and 0 <= j < nB:
                            emit_B_chunk(prev, j)
                    if prev is not None:
                        finish_B(prev)
                    cur = reduce_block(cur)
                    if prev is not None:
                        fire_gather(prev)
                    prev = cur
                # flush any depth-3 remainder (only when NQB == 1)
                while len(pend0) > 1:
                    done = pend0.pop(0)
                    for i in range(len(done[3])):
                        emit_B_chunk(done, i)
                    finish_B(done)
                    fire_gather(done)
                if pend0:
                    prev = pend0.pop(0)
                # tail: final block's B with the remaining out-proj blocks
                # interleaved; the last out-proj's gather-independent half
                # runs before the final gather, the rest after
                nB = len(prev[3])
                for i in range(nB):
                    emit_B_chunk(prev, i)
                    if i == 1 and NQB >= 2:
                        phase3_block(NQB - 2)
                    if i == min(8, nB - 1):
                        phase3_block(NQB - 1, part=0)
                finish_B(prev)
                fire_gather(prev)
                phase3_block(NQB - 1, part=1)

    nc.compile()
    return nc


_NC_CACHE = {}


def get_nc(**cfg):
    key = tuple(sorted(cfg.items()))
    if key not in _NC_CACHE:
        _NC_CACHE[key] = build_nc(**cfg)
    return _NC_CACHE[key]


def _col_index(g):
    p = np.arange(CPC)
    return (p % HD) * HEADS + (HPC * g + p // HD)


def _ow_row_index():
    r = np.arange(D)
    m, p128 = r // P, r % P
    g_, cc = m // NCC, m % NCC
    p256 = cc * P + p128
    lh, hd = p256 // HD, p256 % HD
    return hd * HEADS + (HPC * g_ + lh)


def _split16(w, shi):
    """w -> (fp16(shi*w), e4m3(shi*(w-hi/shi)), e4m3(shi/16*whi))"""
    hi = (shi * w).astype(np.float16)
    lo = shi * w - hi.astype(np.float32)
    l8 = lo.astype(ml_dtypes.float8_e4m3)
    h8 = (hi.astype(np.float32) / 16.0).astype(ml_dtypes.float8_e4m3)
    return hi, l8, h8


def make_in_maps(x, qw, kw, vw, ow, s=S):
    scale = 1.0 / np.sqrt(np.float32(D))
    qws = (qw * scale).astype(np.float32)
    ow_perm = np.ascontiguousarray(ow[_ow_row_index()])

    # x hi/lo splits, shared per batch
    xsplits = []
    for b in range(B):
        xT = np.ascontiguousarray(x[b, :s].T).astype(np.float32)
        xh = (SX * xT).astype(np.float16)          # fp16(16 x)
        xl = SX * xT - xh.astype(np.float32)       # 16 xl
        xdr = np.empty((D, 2, s), dtype=ml_dtypes.float8_e4m3)
        xdr[:, 0, :] = xh.astype(ml_dtypes.float8_e4m3)   # e4m3(16 xh)
        xdr[:, 1, :] = (SXL / SX * xl).astype(ml_dtypes.float8_e4m3)
        xsplits.append((xh, xdr))

    mskst = (-1e10 * np.eye(P, dtype=np.float32)).astype(ml_dtypes.bfloat16)
    mskmv = np.tril(np.ones((P, P), dtype=np.float32), -1).astype(
        ml_dtypes.bfloat16)

    in_maps = []
    for c in range(N_CORES):
        b, g = c // GROUPS, c % GROUPS
        cols = _col_index(g)
        wq = np.ascontiguousarray(qws[:, cols])
        wk = np.ascontiguousarray(kw[:, cols]).astype(np.float32)
        qh, ql8, qh8 = _split16(wq, SWQ)
        kh, kl8, kh8 = _split16(wk, SWK)
        wqdr = np.empty((D, 2, CPC), dtype=ml_dtypes.float8_e4m3)
        wqdr[:, 0, :] = ql8
        wqdr[:, 1, :] = qh8
        wkdr = np.empty((D, 2, CPC), dtype=ml_dtypes.float8_e4m3)
        wkdr[:, 0, :] = kl8
        wkdr[:, 1, :] = kh8
        xh, xdr = xsplits[b]
        in_maps.append({
            "xh16": xh,
            "xdr": xdr,
            "wqh": qh,
            "wqdr": wqdr,
            "wkh": kh,
            "wkdr": wkdr,
            "wvh": (np.ascontiguousarray(vw[:, cols]) / SX).astype(
                np.float16),
            "woh": np.ascontiguousarray(
                ow_perm[:, g * CPC:(g + 1) * CPC]).astype(np.float16),
            "mskst": mskst,
            "mskmv": mskmv,
        })
    return in_maps


def assemble_output(results, s=S):
    out = np.empty((B, s, D), dtype=np.float32)
    for c in range(N_CORES):
        b, g = c // GROUPS, c % GROUPS
        oT = results[c]["outT"]  # [NCC, P, s]
        for occ in range(NCC):
            out[b, :, g * CPC + occ * P:(g * CPC + (occ + 1) * P)] = oT[occ].T
    return out


def run_on_hw(x, qw, kw, vw, ow, trace=False, **cfg_over):
    cfg = dict(DEFAULT_CFG)
    cfg.update(cfg_over)
    s = cfg["s"]
    nc = get_nc(**cfg)
    in_maps = make_in_maps(x, qw, kw, vw, ow, s=s)
    res = run_bass_kernel_spmd(nc, in_maps, core_ids=list(range(N_CORES)),
                               trace=trace)
    return assemble_output(res.results, s=s), res


def kernel(x, qw, kw, vw, ow):
    out, _ = run_on_hw(np.asarray(x, dtype=np.float32),
                       np.asarray(qw, dtype=np.float32),
                       np.asarray(kw, dtype=np.float32),
                       np.asarray(vw, dtype=np.float32),
                       np.asarray(ow, dtype=np.float32))
    return out
